# revision 1
# baseline (speedup 1.0000x reference)
"""TransformerConv 2-layer GNN encoder on 8 Trainium2 NeuronCores.

Strategy (dst-sharded graph parallelism):
  - Nodes are assigned to 8 cores x 20 blocks x 128 slots via degree-balanced
    first-fit-decreasing so every block has <= 1024 incoming edges -> exactly
    8 edge-chunks of 128 edges per block (uniform compile-time schedule).
  - Launch 0: each core computes k1|v1 rows for its own node shard.
  - Host gathers per-edge source rows between launches (pure data movement),
    so the device only ever does sequential DMA.
  - Launch 1: per-edge attention for layer 1 (e = ea@WeT on PE, segment
    softmax via host-built one-hot S matrices contracted on PE, dot products
    via fused DVE tensor_tensor_reduce, exp + weighting on ACT), then the
    layer-2 node-phase matmuls fused into each block epilogue.
  - Launch 2: same edge pipeline at width 64 for layer 2 -> z.
"""

import sys

sys.path.insert(0, "/opt/trn_rl_repo")

import json

import numpy as np

# ----------------------------------------------------------------------------
# Problem constants (hardcoded per contract)
# ----------------------------------------------------------------------------
N, E, IN_DIM, EDGE_DIM, HID, OUT = 20000, 160000, 128, 32, 128, 64
H1 = 4
F1 = H1 * HID  # 512
NCORES = 8
BLKS = 20          # dst blocks per core
BLKN = 128         # nodes per block
NLOC = BLKS * BLKN  # 2560 nodes per core
NTOT = NCORES * NLOC  # 20480 slots
CPB = 8            # chunks per block
T = 128            # edges per chunk
ECHUNKS = BLKS * CPB  # 160 chunks per core
ELOC = ECHUNKS * T    # 20480 edge slots per core

ISQ1 = 1.0 / np.sqrt(np.float32(HID))
ISQ2 = 1.0 / np.sqrt(np.float32(OUT))
DENOM_EPS = 1e-30

# ----------------------------------------------------------------------------
# Walrus single-wait shim + NTFF profiling hook (inlined; must be
# self-contained).  Walrus in this container encodes only ONE sync-wait per
# instruction; Tile emits more.  Split excess waits onto EventSemaphore
# instructions right before the offending instruction.
# ----------------------------------------------------------------------------
_shim_installed = False


def _split_waits_in_bir(bir_bytes: bytes) -> bytes:
    d = json.loads(bir_bytes)
    for fn in d.get("functions", []):
        for blk in fn.get("blocks", []):
            new_insts = []
            for ins in blk.get("instructions", []):
                si = ins.get("sync_info") or {}
                waits = si.get("on_wait") or []
                if len(waits) > 1:
                    for k, w in enumerate(waits[:-1]):
                        ev = {
                            "name": f"{ins['name']}_wsplit{k}",
                            "opcode": "EventSemaphore",
                            "engine": ins["engine"],
                            "ins": [],
                            "outs": [],
                            "sync_info": {"on_wait": [w], "on_update": []},
                        }
                        if "debug" in ins:
                            ev["debug"] = ins["debug"]
                        new_insts.append(ev)
                    si["on_wait"] = [waits[-1]]
                new_insts.append(ins)
            blk["instructions"] = new_insts
    return json.dumps(d).encode()


def _install_shim():
    global _shim_installed
    if _shim_installed:
        return
    import concourse.bass2jax as bass2jax
    import concourse.bass_utils as bass_utils

    orig = bass_utils.compile_bir_kernel

    def wrapped(bir_json, tmpdir, neff_name="file.neff"):
        if isinstance(bir_json, str):
            bir_json = bir_json.encode()
        return orig(_split_waits_in_bir(bir_json), tmpdir, neff_name=neff_name)

    bass_utils.compile_bir_kernel = wrapped
    bass2jax.compile_bir_kernel = wrapped

    # NTFF profile hook (missing antenv.axon_hooks in this image)
    import types

    try:
        from antenv import axon_hooks  # noqa: F401
    except ImportError:
        import antenv

        mod = types.ModuleType("antenv.axon_hooks")
        _state = {"hook": None}
        mod.set_axon_ntff_profile_hook = lambda h: _state.__setitem__("hook", h)
        mod.get_axon_ntff_profile_hook = lambda: _state["hook"]
        sys.modules["antenv.axon_hooks"] = mod
        antenv.axon_hooks = mod
        try:
            from trn_agent_boot.trn_boot import _ntff_profile_via_ctypes

            hook = _ntff_profile_via_ctypes("/opt/axon/libaxon_pjrt.so")
            if hook is not None:
                mod.set_axon_ntff_profile_hook(hook)
        except Exception:
            pass
    _shim_installed = True


# ----------------------------------------------------------------------------
# Host-side graph planning
# ----------------------------------------------------------------------------
class _Plan:
    pass


def _make_plan(ei: np.ndarray) -> _Plan:
    """Assign nodes to (core, block, slot); build per-core edge schedule."""
    src = np.asarray(ei[0], dtype=np.int64)
    dst = np.asarray(ei[1], dtype=np.int64)
    deg = np.bincount(dst, minlength=N)  # in-degree

    nbins = NCORES * BLKS  # 160
    cap_e = CPB * T  # 1024 edges per bin
    cap_n = BLKN  # 128 nodes per bin

    order = np.argsort(-deg, kind="stable")
    bin_e = np.zeros(nbins, dtype=np.int64)
    bin_n = np.zeros(nbins, dtype=np.int64)
    node_bin = np.empty(N, dtype=np.int64)
    # first-fit decreasing over a rotating start to spread load
    start = 0
    for nd in order:
        d = deg[nd]
        placed = False
        for k in range(nbins):
            b = (start + k) % nbins
            if bin_e[b] + d <= cap_e and bin_n[b] < cap_n:
                node_bin[nd] = b
                bin_e[b] += d
                bin_n[b] += 1
                start = (b + 1) % nbins
                placed = True
                break
        if not placed:  # cannot happen for this distribution; fail loudly
            raise RuntimeError("bin packing failed")

    # slot within bin
    node_slot = np.empty(N, dtype=np.int64)
    fill = np.zeros(nbins, dtype=np.int64)
    for nd in range(N):
        b = node_bin[nd]
        node_slot[nd] = fill[b]
        fill[b] += 1

    # global slot id: bins are laid out core-major: bin b -> core b//BLKS
    node_gslot = node_bin * BLKN + node_slot  # in [0, NTOT)

    # per-core edge schedule: edges sorted by (bin, arbitrary), padded per bin
    edge_bin = node_bin[dst]
    eorder = np.argsort(edge_bin, kind="stable")
    sorted_bins = edge_bin[eorder]
    # per-bin edge lists
    bin_starts = np.searchsorted(sorted_bins, np.arange(nbins))
    bin_ends = np.searchsorted(sorted_bins, np.arange(nbins), side="right")

    # per-core arrays of edge slots
    edge_src_gslot = np.zeros((NCORES, ELOC), dtype=np.int64)  # src row to gather
    edge_id = np.full((NCORES, ELOC), -1, dtype=np.int64)  # original edge (or -1 pad)
    edge_dslot = np.full((NCORES, ELOC), -1, dtype=np.int64)  # local dst slot 0..127
    for b in range(nbins):
        core = b // BLKS
        blk = b % BLKS
        s, e = bin_starts[b], bin_ends[b]
        eids = eorder[s:e]
        ne = len(eids)
        base = blk * cap_e
        edge_id[core, base : base + ne] = eids
        edge_src_gslot[core, base : base + ne] = node_gslot[src[eids]]
        edge_dslot[core, base : base + ne] = node_slot[dst[eids]]

    p = _Plan()
    p.node_gslot = node_gslot
    p.edge_src_gslot = edge_src_gslot
    p.edge_id = edge_id
    p.edge_dslot = edge_dslot
    return p



def _build_S_packed(plan):
    """Per-block packed one-hot matrices, bf16.

    S_p[core, b, t, c*BLKN + d]  = 1 if edge (b*CPB+c, t) has dst slot d
    ST_p[core, b, d, c*T + t]    = transpose
    """
    import ml_dtypes

    S = np.zeros((NCORES, ECHUNKS, T, BLKN), dtype=np.float32)
    dslot = plan.edge_dslot.reshape(NCORES, ECHUNKS, T)
    c_idx, ch_idx, t_idx = np.nonzero(dslot >= 0)
    S[c_idx, ch_idx, t_idx, dslot[c_idx, ch_idx, t_idx]] = 1.0
    bf = ml_dtypes.bfloat16
    S_p = np.ascontiguousarray(
        S.reshape(NCORES, BLKS, CPB, T, BLKN).transpose(0, 1, 3, 2, 4)
        .reshape(NCORES, BLKS, T, CPB * BLKN)
    ).astype(bf)
    ST_p = np.ascontiguousarray(
        S.reshape(NCORES, BLKS, CPB, T, BLKN).transpose(0, 1, 4, 2, 3)
        .reshape(NCORES, BLKS, BLKN, CPB * T)
    ).astype(bf)
    return S_p, ST_p


def _build_eaT_packed(plan, ea):
    """eaT_p[core, b, j, c*T + t] edge attrs, bf16, per-block packed."""
    import ml_dtypes

    eid = plan.edge_id.reshape(NCORES, ECHUNKS, T)
    valid = eid >= 0
    gathered = np.zeros((NCORES, ECHUNKS, T, EDGE_DIM), dtype=np.float32)
    gathered[valid] = ea[eid[valid]]
    out = np.ascontiguousarray(
        gathered.reshape(NCORES, BLKS, CPB, T, EDGE_DIM).transpose(0, 1, 4, 2, 3)
        .reshape(NCORES, BLKS, EDGE_DIM, CPB * T)
    ).astype(ml_dtypes.bfloat16)
    return out


def _pack_rows_blocks(rows, width):
    """[cores, ELOC, width] -> [cores, BLKS, T, CPB*width] block-major."""
    return np.ascontiguousarray(
        rows.reshape(NCORES, BLKS, CPB, T, width).transpose(0, 1, 3, 2, 4)
        .reshape(NCORES, BLKS, T, CPB * width)
    )


# ----------------------------------------------------------------------------
# Bass kernel builders
# ----------------------------------------------------------------------------
_built = {}


def _get_nc():
    import concourse.bass as bass

    return bass.Bass(target_bir_lowering=False, trn_type="TRN2")


def _build_l0():
    """Per core: k1|v1 = x_loc @ [Wk1|Wv1]^T + b for the core's nodes (bf16 out)."""
    import concourse.mybir as mybir
    from concourse.tile import TileContext

    dt = mybir.dt
    nc = _get_nc()
    xT = nc.dram_tensor("xT", [IN_DIM, NLOC], dt.float32r, kind="ExternalInput")
    WkvT = nc.dram_tensor("WkvT", [IN_DIM, 2 * F1], dt.float32r, kind="ExternalInput")
    bkv = nc.dram_tensor("bkv", [1, 2 * F1], dt.float32r, kind="ExternalInput")
    ones = nc.dram_tensor("ones", [1, BLKN], dt.float32r, kind="ExternalInput")
    kv = nc.dram_tensor("kv", [NLOC, 2 * F1], dt.bfloat16, kind="ExternalOutput")

    with TileContext(nc) as tc:
        with (
            tc.tile_pool(name="const", bufs=1) as cpool,
            tc.tile_pool(name="sb", bufs=4) as pool,
            tc.tile_pool(name="ps", bufs=4, space="PSUM") as psp,
        ):
            w = cpool.tile([IN_DIM, 2 * F1], dt.float32r)
            nc.sync.dma_start(w[:], WkvT[:])
            bt = cpool.tile([1, 2 * F1], dt.float32r)
            nc.sync.dma_start(bt[:], bkv[:])
            on = cpool.tile([1, BLKN], dt.float32r)
            nc.sync.dma_start(on[:], ones[:])
            xfull = cpool.tile([IN_DIM, NLOC], dt.float32r)
            nc.sync.dma_start(xfull[:], xT[:])
            for b in range(BLKS):
                xt = xfull[:, b * BLKN : (b + 1) * BLKN]
                res = pool.tile([BLKN, 2 * F1], dt.bfloat16, tag="res")
                for half in range(2):
                    lo, hi = half * F1, (half + 1) * F1
                    ps = psp.tile([BLKN, F1], dt.float32, tag="ps")
                    nc.tensor.matmul(ps[:], xt, w[:, lo:hi], start=True, stop=False)
                    nc.tensor.matmul(ps[:], on[:], bt[:, lo:hi], start=False, stop=True)
                    nc.scalar.activation(
                        res[:, lo:hi], ps[:], mybir.ActivationFunctionType.Copy
                    )
                nc.sync.dma_start(kv[b * BLKN : (b + 1) * BLKN, :], res[:])
    return nc


def _build_l1():
    """Edge phase layer 1 + fused layer-2 node phase (bf16 edge path)."""
    import concourse.mybir as mybir
    from concourse.tile import TileContext

    dt = mybir.dt
    nc = _get_nc()
    f32, f32r, bf = dt.float32, dt.float32r, dt.bfloat16

    kvp = nc.dram_tensor("kvp", [BLKS, T, CPB * 2 * F1], bf, kind="ExternalInput")
    Sd = nc.dram_tensor("S", [BLKS, T, CPB * BLKN], bf, kind="ExternalInput")
    STd = nc.dram_tensor("ST", [BLKS, BLKN, CPB * T], bf, kind="ExternalInput")
    eaTd = nc.dram_tensor("eaT", [BLKS, EDGE_DIM, CPB * T], bf, kind="ExternalInput")
    xT = nc.dram_tensor("xT", [IN_DIM, NLOC], f32r, kind="ExternalInput")
    WqT = nc.dram_tensor("WqT", [IN_DIM, F1], f32r, kind="ExternalInput")
    bq = nc.dram_tensor("bq", [1, F1], f32r, kind="ExternalInput")
    WsT = nc.dram_tensor("WsT", [IN_DIM, F1], f32r, kind="ExternalInput")
    bs = nc.dram_tensor("bs", [1, F1], f32r, kind="ExternalInput")
    WeT = nc.dram_tensor("WeT", [EDGE_DIM, F1], bf, kind="ExternalInput")
    W2T = nc.dram_tensor("W2T", [BLKN, 4 * 4 * OUT], f32r, kind="ExternalInput")
    b2 = nc.dram_tensor("b2", [1, 4 * OUT], f32r, kind="ExternalInput")
    ones = nc.dram_tensor("ones", [1, BLKN], f32r, kind="ExternalInput")
    identd = nc.dram_tensor("ident", [BLKN, BLKN], f32, kind="ExternalInput")
    out2 = nc.dram_tensor("out2", [NLOC, 4 * OUT], f32, kind="ExternalOutput")

    AF = mybir.ActivationFunctionType
    ALU = mybir.AluOpType

    with TileContext(nc) as tc:
        with (
            tc.tile_pool(name="const", bufs=1) as cpool,
            tc.tile_pool(name="blk", bufs=3) as bpool,
            tc.tile_pool(name="sb", bufs=4) as pool,
            tc.tile_pool(name="pshold", bufs=1, space="PSUM") as psh,
            tc.tile_pool(name="pschunk", bufs=2, space="PSUM") as psc,
        ):
            wq = cpool.tile([IN_DIM, F1], f32r)
            nc.sync.dma_start(wq[:], WqT[:])
            ws = cpool.tile([IN_DIM, F1], f32r)
            nc.sync.dma_start(ws[:], WsT[:])
            we = cpool.tile([EDGE_DIM, F1], bf)
            nc.sync.dma_start(we[:], WeT[:])
            w2 = cpool.tile([BLKN, 4 * 4 * OUT], f32r)
            nc.sync.dma_start(w2[:], W2T[:])
            bqt = cpool.tile([1, F1], f32r)
            nc.sync.dma_start(bqt[:], bq[:])
            bst = cpool.tile([1, F1], f32r)
            nc.sync.dma_start(bst[:], bs[:])
            b2t = cpool.tile([1, 4 * OUT], f32r)
            nc.sync.dma_start(b2t[:], b2[:])
            on = cpool.tile([1, BLKN], f32r)
            nc.sync.dma_start(on[:], ones[:])
            ident = cpool.tile([BLKN, BLKN], f32)
            nc.sync.dma_start(ident[:], identd[:])

            for b in range(BLKS):
                xt = bpool.tile([IN_DIM, BLKN], f32r, tag="xt")
                nc.sync.dma_start(xt[:], xT[:, b * BLKN : (b + 1) * BLKN])
                # block-batched edge inputs (one DMA each)
                kvb = bpool.tile([T, CPB * 2 * F1], bf, tag="kvb")
                nc.sync.dma_start(kvb[:], kvp[b])
                sb_ = bpool.tile([T, CPB * BLKN], bf, tag="sb_")
                nc.sync.dma_start(sb_[:], Sd[b])
                stb = bpool.tile([BLKN, CPB * T], bf, tag="stb")
                nc.sync.dma_start(stb[:], STd[b])
                eab = bpool.tile([EDGE_DIM, CPB * T], bf, tag="eab")
                nc.sync.dma_start(eab[:], eaTd[b])

                # Q block
                psq = psh.tile([BLKN, F1], f32, tag="scratch")
                nc.tensor.matmul(psq[:], xt[:], wq[:], start=True, stop=False)
                nc.tensor.matmul(psq[:], on[:], bqt[:], start=False, stop=True)
                qblk = bpool.tile([BLKN, F1], bf, tag="qblk")
                nc.scalar.activation(qblk[:], psq[:], AF.Copy)

                psnum = psh.tile([BLKN, F1], f32, tag="psnum")
                psden = psh.tile([BLKN, H1], f32, tag="psden")

                for i in range(CPB):
                    kvk = kvb[:, i * 2 * F1 : i * 2 * F1 + F1]
                    kvv = kvb[:, i * 2 * F1 + F1 : (i + 1) * 2 * F1]
                    s_ = sb_[:, i * BLKN : (i + 1) * BLKN]
                    st_ = stb[:, i * T : (i + 1) * T]
                    eat = eab[:, i * T : (i + 1) * T]

                    pse = psc.tile([T, F1], f32, tag="pse")
                    nc.tensor.matmul(pse[:], eat, we[:], start=True, stop=True)
                    pseb = pool.tile([T, F1], bf, tag="pseb")
                    nc.scalar.activation(pseb[:], pse[:], AF.Copy)
                    psqt = psc.tile([T, F1], f32, tag="psqt")
                    nc.tensor.matmul(psqt[:], st_, qblk[:], start=True, stop=True)
                    qtb = pool.tile([T, F1], bf, tag="qtb")
                    nc.scalar.activation(qtb[:], psqt[:], AF.Copy)

                    kj = pool.tile([T, F1], bf, tag="kj")
                    nc.vector.tensor_tensor(kj[:], kvk, pseb[:], ALU.add)
                    vj = pool.tile([T, F1], bf, tag="vj")
                    nc.vector.tensor_tensor(vj[:], kvv, pseb[:], ALU.add)
                    prod = pool.tile([T, F1], bf, tag="prod")
                    nc.vector.tensor_tensor(prod[:], qtb[:], kj[:], ALU.mult)
                    alpha = pool.tile([T, H1], f32, tag="alpha")
                    nc.vector.tensor_reduce(
                        alpha[:],
                        prod[:].rearrange("p (h c) -> p h c", h=H1),
                        mybir.AxisListType.X,
                        ALU.add,
                    )
                    ex = pool.tile([T, H1], f32, tag="ex")
                    nc.scalar.activation(ex[:], alpha[:], AF.Exp, scale=ISQ1)
                    exb = pool.tile([T, H1], bf, tag="exb")
                    nc.scalar.activation(exb[:], alpha[:], AF.Exp, scale=ISQ1)
                    exv = pool.tile([T, F1], bf, tag="exv")
                    exbc = (
                        exb[:].rearrange("p (h o) -> p h o", h=H1)
                        .broadcast_to([T, H1, HID])
                    )
                    nc.vector.tensor_tensor(
                        exv[:].rearrange("p (h c) -> p h c", h=H1),
                        vj[:].rearrange("p (h c) -> p h c", h=H1),
                        exbc, ALU.mult,
                    )
                    nc.tensor.matmul(
                        psnum[:], s_, exv[:], start=(i == 0), stop=(i == CPB - 1)
                    )
                    nc.tensor.matmul(
                        psden[:], s_, exb[:], start=(i == 0), stop=(i == CPB - 1)
                    )

                # ---- block epilogue ----
                den = pool.tile([BLKN, H1], f32, tag="den")
                nc.vector.tensor_scalar_max(den[:], psden[:], DENOM_EPS)
                rcp = pool.tile([BLKN, H1], f32, tag="rcp")
                nc.vector.reciprocal(rcp[:], den[:])
                attn = pool.tile([BLKN, F1], f32, tag="attn")
                for h in range(H1):
                    sl = slice(h * HID, (h + 1) * HID)
                    nc.scalar.activation(
                        attn[:, sl], psnum[:, sl], AF.Copy, scale=rcp[:, h : h + 1]
                    )
                pskip = psh.tile([BLKN, F1], f32, tag="scratch")
                nc.tensor.matmul(pskip[:], xt[:], ws[:], start=True, stop=False)
                nc.tensor.matmul(pskip[:], on[:], bst[:], start=False, stop=True)
                hpre = pool.tile([BLKN, F1], f32, tag="hpre")
                nc.vector.tensor_tensor(hpre[:], attn[:], pskip[:], ALU.add)
                hrelu = pool.tile([BLKN, F1], f32, tag="hrelu")
                nc.scalar.activation(hrelu[:], hpre[:], AF.Relu)

                # transpose h (4 x 128x128) via PE, then layer-2 node matmuls
                hT = pool.tile([BLKN, F1], f32r, tag="hT")
                for fb in range(4):
                    sl = slice(fb * BLKN, (fb + 1) * BLKN)
                    pst = psh.tile([BLKN, BLKN], f32, tag="scratch")
                    nc.tensor.transpose(pst[:], hrelu[:, sl], ident[:])
                    nc.scalar.activation(hT[:, sl], pst[:], AF.Copy)
                ps2 = psh.tile([BLKN, 4 * OUT], f32, tag="ps2")
                for fb in range(4):
                    nc.tensor.matmul(
                        ps2[:], hT[:, fb * BLKN : (fb + 1) * BLKN],
                        w2[:, fb * 4 * OUT : (fb + 1) * 4 * OUT],
                        start=(fb == 0), stop=False,
                    )
                nc.tensor.matmul(ps2[:], on[:], b2t[:], start=False, stop=True)
                o2 = pool.tile([BLKN, 4 * OUT], f32, tag="o2")
                nc.scalar.activation(o2[:], ps2[:], AF.Copy)
                nc.sync.dma_start(out2[b * BLKN : (b + 1) * BLKN, :], o2[:])
    return nc


def _build_l2():
    """Edge phase layer 2: z = attn2 + s2 (bf16 edge path, fused denom)."""
    import concourse.mybir as mybir
    from concourse.tile import TileContext

    dt = mybir.dt
    nc = _get_nc()
    f32, f32r, bf = dt.float32, dt.float32r, dt.bfloat16
    D2 = 2 * OUT  # 128: k2|v2 row width

    kvp = nc.dram_tensor("kvp", [BLKS, T, CPB * D2], bf, kind="ExternalInput")
    Sd = nc.dram_tensor("S", [BLKS, T, CPB * BLKN], bf, kind="ExternalInput")
    STd = nc.dram_tensor("ST", [BLKS, BLKN, CPB * T], bf, kind="ExternalInput")
    eaTd = nc.dram_tensor("eaT", [BLKS, EDGE_DIM, CPB * T], bf, kind="ExternalInput")
    q2d = nc.dram_tensor("q2", [NLOC, OUT], bf, kind="ExternalInput")
    s2d = nc.dram_tensor("s2", [NLOC, OUT], f32, kind="ExternalInput")
    WeT = nc.dram_tensor("WeT", [EDGE_DIM, OUT], bf, kind="ExternalInput")
    z = nc.dram_tensor("z", [NLOC, OUT], f32, kind="ExternalOutput")

    AF = mybir.ActivationFunctionType
    ALU = mybir.AluOpType

    with TileContext(nc) as tc:
        with (
            tc.tile_pool(name="const", bufs=1) as cpool,
            tc.tile_pool(name="blk", bufs=3) as bpool,
            tc.tile_pool(name="sb", bufs=4) as pool,
            tc.tile_pool(name="pshold", bufs=1, space="PSUM") as psh,
            tc.tile_pool(name="pschunk", bufs=2, space="PSUM") as psc,
        ):
            we = cpool.tile([EDGE_DIM, OUT], bf)
            nc.sync.dma_start(we[:], WeT[:])
            for b in range(BLKS):
                q2b = bpool.tile([BLKN, OUT], bf, tag="q2b")
                nc.sync.dma_start(q2b[:], q2d[b * BLKN : (b + 1) * BLKN, :])
                s2b = bpool.tile([BLKN, OUT], f32, tag="s2b")
                nc.sync.dma_start(s2b[:], s2d[b * BLKN : (b + 1) * BLKN, :])
                kvb = bpool.tile([T, CPB * D2], bf, tag="kvb")
                nc.sync.dma_start(kvb[:], kvp[b])
                sb_ = bpool.tile([T, CPB * BLKN], bf, tag="sb_")
                nc.sync.dma_start(sb_[:], Sd[b])
                stb = bpool.tile([BLKN, CPB * T], bf, tag="stb")
                nc.sync.dma_start(stb[:], STd[b])
                eab = bpool.tile([EDGE_DIM, CPB * T], bf, tag="eab")
                nc.sync.dma_start(eab[:], eaTd[b])

                # psnum has OUT num cols + 1 denom col
                psnum = psh.tile([BLKN, OUT + 1], f32, tag="psnum")

                for i in range(CPB):
                    kvk = kvb[:, i * D2 : i * D2 + OUT]
                    kvv = kvb[:, i * D2 + OUT : (i + 1) * D2]
                    s_ = sb_[:, i * BLKN : (i + 1) * BLKN]
                    st_ = stb[:, i * T : (i + 1) * T]
                    eat = eab[:, i * T : (i + 1) * T]

                    pse = psc.tile([T, OUT], f32, tag="pse")
                    nc.tensor.matmul(pse[:], eat, we[:], start=True, stop=True)
                    psqt = psc.tile([T, OUT], f32, tag="psqt")
                    nc.tensor.matmul(psqt[:], st_, q2b[:], start=True, stop=True)

                    kj = pool.tile([T, OUT], bf, tag="kj")
                    nc.vector.tensor_tensor(kj[:], kvk, pse[:], ALU.add)
                    vj = pool.tile([T, OUT], bf, tag="vj")
                    nc.vector.tensor_tensor(vj[:], kvv, pse[:], ALU.add)
                    prod = pool.tile([T, OUT], bf, tag="prod")
                    nc.vector.tensor_tensor(prod[:], psqt[:], kj[:], ALU.mult)
                    alpha = pool.tile([T, 1], f32, tag="alpha")
                    nc.vector.tensor_reduce(
                        alpha[:], prod[:], mybir.AxisListType.X, ALU.add
                    )
                    ex = pool.tile([T, 1], f32, tag="ex")
                    nc.scalar.activation(ex[:], alpha[:], AF.Exp, scale=ISQ2)
                    # exv_aug: [exv | ex] so one matmul gives num and denom
                    exv = pool.tile([T, OUT + 1], bf, tag="exv")
                    nc.scalar.activation(exv[:, :OUT], vj[:], AF.Copy, scale=ex[:])
                    nc.scalar.activation(exv[:, OUT:], alpha[:], AF.Exp, scale=ISQ2)
                    nc.tensor.matmul(
                        psnum[:], s_, exv[:], start=(i == 0), stop=(i == CPB - 1)
                    )

                den = pool.tile([BLKN, 1], f32, tag="den")
                nc.vector.tensor_scalar_max(den[:], psnum[:, OUT:], DENOM_EPS)
                rcp = pool.tile([BLKN, 1], f32, tag="rcp")
                nc.vector.reciprocal(rcp[:], den[:])
                attn = pool.tile([BLKN, OUT], f32, tag="attn")
                nc.scalar.activation(attn[:], psnum[:, :OUT], AF.Copy, scale=rcp[:])
                zb = pool.tile([BLKN, OUT], f32, tag="zb")
                nc.vector.tensor_tensor(zb[:], attn[:], s2b[:], ALU.add)
                nc.sync.dma_start(z[b * BLKN : (b + 1) * BLKN, :], zb[:])
    return nc


# ----------------------------------------------------------------------------
# Kernel entry point
# ----------------------------------------------------------------------------
PROFILE = False  # set True (e.g. from test.py) to collect NTFF timing
LAST_EXEC_NS = None
LAST_TRACES = None


def kernel(**inputs):
    global LAST_EXEC_NS, LAST_TRACES
    _install_shim()
    import ml_dtypes

    from concourse import bass_utils

    bf = ml_dtypes.bfloat16

    def _run(nc, in_maps):
        r = bass_utils.run_bass_kernel_spmd(
            nc, in_maps, core_ids=list(range(NCORES)), trace=PROFILE
        )
        if PROFILE:
            _exec_ns.append(r.exec_time_ns)
            _traces.append(r.instructions_and_trace)
        return r

    _exec_ns, _traces = [], []

    x = np.asarray(inputs["x"], dtype=np.float32)
    ei = np.asarray(inputs["ei"])
    ea = np.asarray(inputs["ea"], dtype=np.float32)
    W = {k: np.asarray(v, dtype=np.float32) for k, v in inputs.items()
         if k not in ("x", "ei", "ea")}

    plan = _make_plan(ei)
    S_p, ST_p = _build_S_packed(plan)
    eaT_p = _build_eaT_packed(plan, ea)

    # node features in slot order
    x_slots = np.zeros((NTOT, IN_DIM), dtype=np.float32)
    x_slots[plan.node_gslot] = x
    xT_all = np.ascontiguousarray(x_slots.T)  # [128, NTOT]

    ones = np.ones((1, BLKN), dtype=np.float32)

    # ---------------- launch 0: k1|v1 table ----------------
    if "l0" not in _built:
        _built["l0"] = _build_l0()
    Wkv = np.concatenate([W["Wk1"], W["Wv1"]], axis=0)  # [1024, 128]
    bkv = np.concatenate([W["bk1"], W["bv1"]])[None, :]
    in_maps0 = []
    for c in range(NCORES):
        in_maps0.append({
            "xT": np.ascontiguousarray(xT_all[:, c * NLOC : (c + 1) * NLOC]),
            "WkvT": np.ascontiguousarray(Wkv.T),
            "bkv": bkv,
            "ones": ones,
        })
    r0 = _run(_built["l0"], in_maps0)
    kv1_all = np.concatenate([r0.results[c]["kv"] for c in range(NCORES)], axis=0)

    # host gather of per-edge source rows (bf16), packed per block
    kvrows = kv1_all[plan.edge_src_gslot.reshape(-1)].reshape(
        NCORES, ELOC, 2 * F1
    )
    kvp = _pack_rows_blocks(kvrows, 2 * F1)  # [cores, BLKS, T, CPB*1024] bf16

    # ---------------- launch 1 ----------------
    if "l1" not in _built:
        _built["l1"] = _build_l1()
    W2 = np.concatenate([W["Wk2"], W["Wv2"], W["Wq2"], W["Ws2"]], axis=0)
    b2 = np.concatenate([W["bk2"], W["bv2"], W["bq2"], W["bs2"]])[None, :]
    in_maps1 = []
    for c in range(NCORES):
        in_maps1.append({
            "kvp": kvp[c],
            "S": S_p[c], "ST": ST_p[c], "eaT": eaT_p[c],
            "xT": np.ascontiguousarray(xT_all[:, c * NLOC : (c + 1) * NLOC]),
            "WqT": np.ascontiguousarray(W["Wq1"].T),
            "bq": W["bq1"][None, :],
            "WsT": np.ascontiguousarray(W["Ws1"].T),
            "bs": W["bs1"][None, :],
            "WeT": np.ascontiguousarray(W["We1"].T).astype(bf),
            "W2T": np.ascontiguousarray(
                W2.T.reshape(4, BLKN, 4 * OUT).transpose(1, 0, 2).reshape(BLKN, -1)
            ),
            "b2": b2,
            "ones": ones,
            "ident": np.eye(BLKN, dtype=np.float32),
        })
    r1 = _run(_built["l1"], in_maps1)
    out2_all = np.concatenate([r1.results[c]["out2"] for c in range(NCORES)], axis=0)
    # [NTOT, 256] = [k2 | v2 | q2 | s2]
    kv2_all = out2_all[:, : 2 * OUT]
    q2_all = out2_all[:, 2 * OUT : 3 * OUT]
    s2_all = out2_all[:, 3 * OUT :]

    kv2rows = kv2_all[plan.edge_src_gslot.reshape(-1)].reshape(
        NCORES, ELOC, 2 * OUT
    ).astype(bf)
    kv2p = _pack_rows_blocks(kv2rows, 2 * OUT)

    # ---------------- launch 2 ----------------
    if "l2" not in _built:
        _built["l2"] = _build_l2()
    in_maps2 = []
    for c in range(NCORES):
        in_maps2.append({
            "kvp": kv2p[c],
            "S": S_p[c], "ST": ST_p[c], "eaT": eaT_p[c],
            "q2": np.ascontiguousarray(q2_all[c * NLOC : (c + 1) * NLOC]).astype(bf),
            "s2": np.ascontiguousarray(s2_all[c * NLOC : (c + 1) * NLOC]),
            "WeT": np.ascontiguousarray(W["We2"].T).astype(bf),
        })
    r2 = _run(_built["l2"], in_maps2)
    z_all = np.concatenate([r2.results[c]["z"] for c in range(NCORES)], axis=0)

    z = z_all[plan.node_gslot]  # back to original node order
    if PROFILE:
        LAST_EXEC_NS = sum(int(t) for t in _exec_ns if t) if all(_exec_ns) else None
        LAST_TRACES = _traces
    return z.astype(np.float32)



# revision 18
# speedup vs baseline: 1.3777x; 1.3777x over previous
"""TransformerConv 2-layer GNN encoder on 8 Trainium2 NeuronCores.

Strategy (dst-sharded graph parallelism, v2):
  - Nodes assigned to 8 cores x 20 blocks x 128 slots (degree-balanced FFD);
    each block has <= 1024 incoming edges -> 8 chunks of 128 edges.
  - Launch 0 (node1): feature-major GEMM nodeT = Wn^T @ x per core producing
    k|v|q|qe'|s rows, biases folded in via per-partition ACT/DVE bias.
    qe' = q @ [We_h | bk_h] folds the edge-attr term and k-bias of the
    attention logit into a 33-wide per-node vector (alpha = q.k + qe'.ea').
  - Host gathers per-edge source rows [k_h|ea']x4 | [v_h|ea']x4 (pure data
    movement between launches).
  - Launch 1 (edge1): per chunk: PE gathers [q|qe'] rows via one-hot ST
    matmuls, DVE tensor_tensor_reduce computes alpha per head straight from
    PSUM, ACT exponentiates and scales [v_h|ea'] by ex, PE scatters via
    one-hot S into numerator + 33-wide [ex*ea'|ex] accumulators. Block
    epilogue expands the ea'-accumulator through [We;bv] (the ones column
    doubles as softmax denominator and v-bias), normalizes, adds skip, relu,
    and runs the fused layer-2 node GEMM producing k2|v2|q2|qe2'|s2.
  - Launch 2 (edge2): same edge pipeline at width 64/33 -> z.
"""

import sys

sys.path.insert(0, "/opt/trn_rl_repo")

import json

import numpy as np

# ----------------------------------------------------------------------------
# Problem constants (hardcoded per contract)
# ----------------------------------------------------------------------------
N, E, IN_DIM, EDGE_DIM, HID, OUT = 20000, 160000, 128, 32, 128, 64
H1 = 4
F1 = H1 * HID  # 512
NCORES = 8
BLKS = 20          # dst blocks per core
BLKN = 128         # nodes per block
NLOC = BLKS * BLKN  # 2560 nodes per core
NTOT = NCORES * NLOC  # 20480 slots
CPB = 8            # chunks per block
T = 128            # edges per chunk
ECHUNKS = BLKS * CPB  # 160 chunks per core
ELOC = ECHUNKS * T    # 20480 edge slots per core

EAW = EDGE_DIM + 1          # 33: [ea | 1]
HW1 = HID + EAW             # 161: per-head [k|ea'] / [q|qe'] width
R1W = 2 * H1 * HW1          # 1288 gathered row width, layer 1
QQE1 = H1 * HW1             # 644
NF0 = 2304                  # node1 output rows (2180 used, padded to 18*128)
NCH0 = NF0 // 128           # 18

HW2 = OUT + EAW             # 97
R2W = 2 * HW2               # 194
O2W = 4 * OUT + EAW         # 289: k2|v2|q2|qe2'|s2

ISQ1 = 1.0 / np.sqrt(np.float32(HID))
ISQ2 = 1.0 / np.sqrt(np.float32(OUT))
DENOM_EPS = 1e-30

# ----------------------------------------------------------------------------
# Walrus single-wait shim + NTFF profiling hook (inlined; must be
# self-contained).
# ----------------------------------------------------------------------------
_shim_installed = False


def _split_waits_in_bir(bir_bytes: bytes) -> bytes:
    d = json.loads(bir_bytes)
    for fn in d.get("functions", []):
        for blk in fn.get("blocks", []):
            new_insts = []
            for ins in blk.get("instructions", []):
                si = ins.get("sync_info") or {}
                waits = si.get("on_wait") or []
                if len(waits) > 1:
                    for k, w in enumerate(waits[:-1]):
                        ev = {
                            "name": f"{ins['name']}_wsplit{k}",
                            "opcode": "EventSemaphore",
                            "engine": ins["engine"],
                            "ins": [],
                            "outs": [],
                            "sync_info": {"on_wait": [w], "on_update": []},
                        }
                        if "debug" in ins:
                            ev["debug"] = ins["debug"]
                        new_insts.append(ev)
                    si["on_wait"] = [waits[-1]]
                new_insts.append(ins)
            blk["instructions"] = new_insts
    return json.dumps(d).encode()


def _install_shim():
    global _shim_installed
    if _shim_installed:
        return
    import concourse.bass2jax as bass2jax
    import concourse.bass_utils as bass_utils

    orig = bass_utils.compile_bir_kernel

    def wrapped(bir_json, tmpdir, neff_name="file.neff"):
        if isinstance(bir_json, str):
            bir_json = bir_json.encode()
        return orig(_split_waits_in_bir(bir_json), tmpdir, neff_name=neff_name)

    bass_utils.compile_bir_kernel = wrapped
    bass2jax.compile_bir_kernel = wrapped

    import types

    try:
        from antenv import axon_hooks  # noqa: F401
    except ImportError:
        import antenv

        mod = types.ModuleType("antenv.axon_hooks")
        _state = {"hook": None}
        mod.set_axon_ntff_profile_hook = lambda h: _state.__setitem__("hook", h)
        mod.get_axon_ntff_profile_hook = lambda: _state["hook"]
        sys.modules["antenv.axon_hooks"] = mod
        antenv.axon_hooks = mod
        try:
            from trn_agent_boot.trn_boot import _ntff_profile_via_ctypes

            hook = _ntff_profile_via_ctypes("/opt/axon/libaxon_pjrt.so")
            if hook is not None:
                mod.set_axon_ntff_profile_hook(hook)
        except Exception:
            pass
    _shim_installed = True


# ----------------------------------------------------------------------------
# Host-side graph planning
# ----------------------------------------------------------------------------
class _Plan:
    pass


def _make_plan(ei: np.ndarray) -> _Plan:
    """Assign nodes to (core, block, slot); build per-core edge schedule."""
    src = np.asarray(ei[0], dtype=np.int64)
    dst = np.asarray(ei[1], dtype=np.int64)
    deg = np.bincount(dst, minlength=N)  # in-degree

    nbins = NCORES * BLKS  # 160
    cap_e = CPB * T  # 1024 edges per bin
    cap_n = BLKN  # 128 nodes per bin

    order = np.argsort(-deg, kind="stable")
    bin_e = np.zeros(nbins, dtype=np.int64)
    bin_n = np.zeros(nbins, dtype=np.int64)
    node_bin = np.empty(N, dtype=np.int64)
    start = 0
    for nd in order:
        d = deg[nd]
        placed = False
        for k in range(nbins):
            b = (start + k) % nbins
            if bin_e[b] + d <= cap_e and bin_n[b] < cap_n:
                node_bin[nd] = b
                bin_e[b] += d
                bin_n[b] += 1
                start = (b + 1) % nbins
                placed = True
                break
        if not placed:
            raise RuntimeError("bin packing failed")

    node_slot = np.empty(N, dtype=np.int64)
    fill = np.zeros(nbins, dtype=np.int64)
    for nd in range(N):
        b = node_bin[nd]
        node_slot[nd] = fill[b]
        fill[b] += 1

    node_gslot = node_bin * BLKN + node_slot  # in [0, NTOT)

    edge_bin = node_bin[dst]
    eorder = np.argsort(edge_bin, kind="stable")
    sorted_bins = edge_bin[eorder]
    bin_starts = np.searchsorted(sorted_bins, np.arange(nbins))
    bin_ends = np.searchsorted(sorted_bins, np.arange(nbins), side="right")

    edge_src_gslot = np.zeros((NCORES, ELOC), dtype=np.int64)
    edge_id = np.full((NCORES, ELOC), -1, dtype=np.int64)
    edge_dslot = np.full((NCORES, ELOC), -1, dtype=np.int64)
    for b in range(nbins):
        core = b // BLKS
        blk = b % BLKS
        s, e = bin_starts[b], bin_ends[b]
        eids = eorder[s:e]
        ne = len(eids)
        base = blk * cap_e
        edge_id[core, base : base + ne] = eids
        edge_src_gslot[core, base : base + ne] = node_gslot[src[eids]]
        edge_dslot[core, base : base + ne] = node_slot[dst[eids]]

    p = _Plan()
    p.node_gslot = node_gslot
    p.edge_src_gslot = edge_src_gslot
    p.edge_id = edge_id
    p.edge_dslot = edge_dslot
    return p


def _build_S_packed(plan):
    """Per-block packed one-hot matrices, bf16."""
    import ml_dtypes

    S = np.zeros((NCORES, ECHUNKS, T, BLKN), dtype=np.float32)
    dslot = plan.edge_dslot.reshape(NCORES, ECHUNKS, T)
    c_idx, ch_idx, t_idx = np.nonzero(dslot >= 0)
    S[c_idx, ch_idx, t_idx, dslot[c_idx, ch_idx, t_idx]] = 1.0
    bf = ml_dtypes.bfloat16
    S_p = np.ascontiguousarray(
        S.reshape(NCORES, BLKS, CPB, T, BLKN).transpose(0, 1, 3, 2, 4)
        .reshape(NCORES, BLKS, T, CPB * BLKN)
    ).astype(bf)
    ST_p = np.ascontiguousarray(
        S.reshape(NCORES, BLKS, CPB, T, BLKN).transpose(0, 1, 4, 2, 3)
        .reshape(NCORES, BLKS, BLKN, CPB * T)
    ).astype(bf)
    return S_p, ST_p


def _pack_rows_blocks(rows, width):
    """[cores, ELOC, width] -> [cores, BLKS, T, CPB*width] block-major."""
    return np.ascontiguousarray(
        rows.reshape(NCORES, BLKS, CPB, T, width).transpose(0, 1, 3, 2, 4)
        .reshape(NCORES, BLKS, T, CPB * width)
    )


# ----------------------------------------------------------------------------
# Bass kernel builders
# ----------------------------------------------------------------------------
_built = {}


def _get_nc():
    import concourse.bass as bass

    return bass.Bass(target_bir_lowering=False, trn_type="TRN2")


def _build_node1():
    """Feature-major node GEMM: nodeT[f, n] = sum_d Wn[d, f] x[n, d] + bias[f]."""
    import concourse.mybir as mybir
    from concourse.tile import TileContext

    dt = mybir.dt
    nc = _get_nc()
    f32, bf = dt.float32, dt.bfloat16
    AF = mybir.ActivationFunctionType

    xT = nc.dram_tensor("xT", [IN_DIM, NLOC], bf, kind="ExternalInput")
    Wn = nc.dram_tensor("Wn", [IN_DIM, NF0], bf, kind="ExternalInput")
    bT = nc.dram_tensor("bT", [128, NCH0], f32, kind="ExternalInput")
    nodeT = nc.dram_tensor("nodeT", [NF0, NLOC], bf, kind="ExternalOutput")

    with TileContext(nc) as tc:
        with (
            tc.tile_pool(name="const", bufs=1) as cpool,
            tc.tile_pool(name="ob", bufs=2) as opool,
            tc.tile_pool(name="ps", bufs=6, space="PSUM") as psp,
        ):
            w = cpool.tile([IN_DIM, NF0], bf)
            nc.sync.dma_start(w[:], Wn[:])
            bt = cpool.tile([128, NCH0], f32)
            nc.sync.dma_start(bt[:], bT[:])
            xfull = cpool.tile([IN_DIM, NLOC], bf)
            nc.sync.dma_start(xfull[:], xT[:])
            for fc in range(NCH0):
                wslice = w[:, fc * 128 : (fc + 1) * 128]
                ob = opool.tile([128, NLOC], bf, tag="ob")
                for g in range(NLOC // 512):
                    ps = psp.tile([128, 512], f32, tag="ps")
                    nc.tensor.matmul(
                        ps[:], wslice, xfull[:, g * 512 : (g + 1) * 512],
                        start=True, stop=True,
                    )
                    osl = ob[:, g * 512 : (g + 1) * 512]
                    if g % 3 == 2:
                        nc.vector.tensor_scalar_add(osl, ps[:], bt[:, fc : fc + 1])
                    else:
                        nc.scalar.activation(
                            osl, ps[:], AF.Identity, bias=bt[:, fc : fc + 1]
                        )
                nc.sync.dma_start(nodeT[fc * 128 : (fc + 1) * 128, :], ob[:])
    return nc


def _build_edge1():
    """Layer-1 edge phase -> h (relu'd layer-1 output), node-major."""
    import concourse.mybir as mybir
    from concourse.tile import TileContext

    dt = mybir.dt
    nc = _get_nc()
    f32, bf = dt.float32, dt.bfloat16
    AF = mybir.ActivationFunctionType
    ALU = mybir.AluOpType

    qqed = nc.dram_tensor("qqe", [NLOC, QQE1], bf, kind="ExternalInput")
    s1d = nc.dram_tensor("s1", [NLOC, F1], bf, kind="ExternalInput")
    Rd = nc.dram_tensor("R", [BLKS, T, CPB * R1W], bf, kind="ExternalInput")
    Sd = nc.dram_tensor("S", [BLKS, T, CPB * BLKN], bf, kind="ExternalInput")
    STd = nc.dram_tensor("ST", [BLKS, BLKN, CPB * T], bf, kind="ExternalInput")
    Wevd = nc.dram_tensor("Wevp", [2 * EAW, 2 * 256], bf, kind="ExternalInput")
    identd = nc.dram_tensor("ident", [BLKN, BLKN], bf, kind="ExternalInput")
    hd = nc.dram_tensor("h", [NLOC, F1], bf, kind="ExternalOutput")

    with TileContext(nc) as tc:
        with (
            tc.tile_pool(name="const", bufs=1) as cpool,
            tc.tile_pool(name="blk", bufs=2) as bpool,
            tc.tile_pool(name="sb", bufs=3) as pool,
            tc.tile_pool(name="epi", bufs=2) as epool,
            tc.tile_pool(name="psq", bufs=2, space="PSUM") as psq,
            tc.tile_pool(name="psacc", bufs=1, space="PSUM") as psa,
            tc.tile_pool(name="psepi", bufs=2, space="PSUM") as pse,
        ):
            wevp = cpool.tile([2 * EAW, 2 * 256], bf)
            nc.sync.dma_start(wevp[:], Wevd[:])
            ident = cpool.tile([BLKN, BLKN], bf)
            nc.sync.dma_start(ident[:], identd[:])

            for b in range(BLKS):
                qqe_b = bpool.tile([BLKN, QQE1], bf, tag="qqe_b")
                nc.sync.dma_start(qqe_b[:], qqed[b * BLKN : (b + 1) * BLKN, :])
                s1b = bpool.tile([BLKN, F1], bf, tag="s1b")
                nc.sync.dma_start(s1b[:], s1d[b * BLKN : (b + 1) * BLKN, :])
                Sb = bpool.tile([T, CPB * BLKN], bf, tag="Sb")
                nc.sync.dma_start(Sb[:], Sd[b])
                STb = bpool.tile([BLKN, CPB * T], bf, tag="STb")
                nc.sync.dma_start(STb[:], STd[b])
                Rb = bpool.tile([T, CPB * R1W], bf, tag="Rb")
                nc.sync.dma_start(Rb[:], Rd[b])

                psN = psa.tile([BLKN, F1], f32, tag="psN")
                psE = psa.tile([BLKN, H1 * EAW], f32, tag="psE")

                for i in range(CPB):
                    Rc = Rb[:, i * R1W : (i + 1) * R1W]
                    s_ = Sb[:, i * BLKN : (i + 1) * BLKN]
                    st_ = STb[:, i * T : (i + 1) * T]

                    qtA = psq.tile([T, 3 * HW1], f32, tag="qtA")
                    nc.tensor.matmul(
                        qtA[:], st_, qqe_b[:, : 3 * HW1], start=True, stop=True
                    )
                    qtB = psq.tile([T, HW1], f32, tag="qtB")
                    nc.tensor.matmul(
                        qtB[:], st_, qqe_b[:, 3 * HW1 : QQE1], start=True, stop=True
                    )

                    # copy gathered [q|qe'] to SBUF bf16 (ACT + DVE split)
                    qtqeb = pool.tile([T, QQE1], bf, tag="qtqeb")
                    nc.scalar.activation(qtqeb[:, : 3 * HW1], qtA[:], AF.Copy)
                    nc.vector.tensor_scalar_add(qtqeb[:, 3 * HW1 :], qtB[:], 0.0)

                    prod = pool.tile([T, QQE1], bf, tag="prod")
                    nc.vector.tensor_tensor(
                        prod[:], qtqeb[:], Rc[:, :QQE1], ALU.mult
                    )
                    alpha = pool.tile([T, H1], f32, tag="alpha")
                    nc.vector.tensor_reduce(
                        alpha[:],
                        prod[:].rearrange("p (h w) -> p h w", h=H1),
                        mybir.AxisListType.X,
                        ALU.add,
                    )
                    ex4 = pool.tile([T, H1], f32, tag="ex4")
                    nc.scalar.activation(ex4[:], alpha[:], AF.Exp, scale=ISQ1)

                    EXV = pool.tile([T, QQE1], bf, tag="EXV")
                    for h in range(H1):
                        vsl = Rc[:, QQE1 + h * HW1 : QQE1 + (h + 1) * HW1]
                        esl = EXV[:, h * HW1 : (h + 1) * HW1]
                        exh = ex4[:, h : h + 1]
                        if h == 1:
                            nc.vector.tensor_scalar_mul(esl, vsl, exh)
                        else:
                            nc.scalar.activation(esl, vsl, AF.Copy, scale=exh)
                    ev3 = EXV[:].rearrange("p (h w) -> p h w", h=H1)
                    nc.tensor.matmul(
                        psN[:], s_, ev3[:, :, :HID],
                        start=(i == 0), stop=False,
                    )
                    nc.tensor.matmul(
                        psE[:], s_, ev3[:, :, HID:],
                        start=(i == 0), stop=(i == CPB - 1),
                    )

                # ---- block epilogue ----
                eEb = epool.tile([BLKN, H1 * EAW], bf, tag="eEb")
                nc.scalar.activation(eEb[:], psE[:], AF.Copy)
                den = epool.tile([BLKN, H1], f32, tag="den")
                nc.vector.tensor_scalar_max(
                    den[:].rearrange("p (h o) -> p h o", o=1),
                    psE[:].rearrange("p (h w) -> p h w", h=H1)[:, :, EDGE_DIM : EDGE_DIM + 1],
                    DENOM_EPS,
                )
                rcp = epool.tile([BLKN, H1], f32, tag="rcp")
                nc.vector.reciprocal(rcp[:], den[:])

                for p in range(2):
                    pst = pse.tile([2 * EAW, BLKN], bf, tag="pst")
                    nc.tensor.transpose(
                        pst[:], eEb[:, p * 2 * EAW : (p + 1) * 2 * EAW], ident[:]
                    )
                    eET = epool.tile([2 * EAW, BLKN], bf, tag="eET")
                    nc.vector.tensor_scalar_add(eET[:], pst[:], 0.0)
                    nc.tensor.matmul(
                        psN[:, p * 256 : (p + 1) * 256],
                        eET[:],
                        wevp[:, p * 256 : (p + 1) * 256],
                        start=False,
                        stop=True,
                        skip_group_check=True,
                    )

                hb = epool.tile([BLKN, F1], bf, tag="hb")
                for h in range(H1):
                    nc.vector.scalar_tensor_tensor(
                        hb[:, h * HID : (h + 1) * HID],
                        psN[:, h * HID : (h + 1) * HID],
                        rcp[:, h : h + 1],
                        s1b[:, h * HID : (h + 1) * HID],
                        ALU.mult,
                        ALU.add,
                    )
                hr = epool.tile([BLKN, F1], bf, tag="hr")
                nc.vector.tensor_scalar_max(hr[:], hb[:], 0.0)
                nc.sync.dma_start(hd[b * BLKN : (b + 1) * BLKN, :], hr[:])
    return nc


def _build_gemm2():
    """Feature-major layer-2 node GEMM: out2T = W2cat @ h^T + b2."""
    import concourse.mybir as mybir
    from concourse.tile import TileContext

    dt = mybir.dt
    nc = _get_nc()
    f32, bf = dt.float32, dt.bfloat16
    AF = mybir.ActivationFunctionType

    hTd = nc.dram_tensor("hT", [F1, NLOC], bf, kind="ExternalInput")
    W2d = nc.dram_tensor("W2a", [128, 4 * O2W], bf, kind="ExternalInput")
    b2Td = nc.dram_tensor("b2T", [128, 3], f32, kind="ExternalInput")
    out2T = nc.dram_tensor("out2T", [O2W, NLOC], bf, kind="ExternalOutput")

    FCS = [(0, 128), (128, 256), (256, O2W)]
    with TileContext(nc) as tc:
        with (
            tc.tile_pool(name="const", bufs=1) as cpool,
            tc.tile_pool(name="ob", bufs=2) as opool,
            tc.tile_pool(name="ps", bufs=6, space="PSUM") as psp,
        ):
            w2a = cpool.tile([128, 4 * O2W], bf)
            nc.sync.dma_start(w2a[:], W2d[:])
            b2t = cpool.tile([128, 3], f32)
            nc.sync.dma_start(b2t[:], b2Td[:])
            hts = []
            for hk in range(4):
                ht = cpool.tile([128, NLOC], bf, name=f"ht{hk}")
                nc.sync.dma_start(ht[:], hTd[hk * 128 : (hk + 1) * 128, :])
                hts.append(ht)
            for ci, (f0, f1) in enumerate(FCS):
                fw = f1 - f0
                ob = opool.tile([128, NLOC], bf, tag="ob")
                for g in range(NLOC // 512):
                    ps = psp.tile([128, 512], f32, tag="ps")
                    for hk in range(4):
                        nc.tensor.matmul(
                            ps[:fw, :],
                            w2a[:, hk * O2W + f0 : hk * O2W + f1],
                            hts[hk][:, g * 512 : (g + 1) * 512],
                            start=(hk == 0), stop=(hk == 3),
                        )
                    osl = ob[:fw, g * 512 : (g + 1) * 512]
                    if g % 3 == 2:
                        nc.vector.tensor_scalar_add(
                            osl, ps[:fw, :], b2t[:fw, ci : ci + 1]
                        )
                    else:
                        nc.scalar.activation(
                            osl, ps[:fw, :], AF.Identity,
                            bias=b2t[:fw, ci : ci + 1],
                        )
                nc.sync.dma_start(out2T[f0:f1, :], ob[:fw, :])
    return nc


def _build_edge2():
    """Layer-2 edge phase -> z."""
    import concourse.mybir as mybir
    from concourse.tile import TileContext

    dt = mybir.dt
    nc = _get_nc()
    f32, bf = dt.float32, dt.bfloat16
    AF = mybir.ActivationFunctionType
    ALU = mybir.AluOpType

    qqed = nc.dram_tensor("qqe2", [NLOC, HW2], bf, kind="ExternalInput")
    s2d = nc.dram_tensor("s2", [NLOC, OUT], bf, kind="ExternalInput")
    Rd = nc.dram_tensor("R2", [BLKS, T, CPB * R2W], bf, kind="ExternalInput")
    Sd = nc.dram_tensor("S", [BLKS, T, CPB * BLKN], bf, kind="ExternalInput")
    STd = nc.dram_tensor("ST", [BLKS, BLKN, CPB * T], bf, kind="ExternalInput")
    Wevd = nc.dram_tensor("Wev2", [EAW, OUT], bf, kind="ExternalInput")
    identd = nc.dram_tensor("ident", [BLKN, BLKN], bf, kind="ExternalInput")
    zd = nc.dram_tensor("z", [NLOC, OUT], f32, kind="ExternalOutput")

    with TileContext(nc) as tc:
        with (
            tc.tile_pool(name="const", bufs=1) as cpool,
            tc.tile_pool(name="blk", bufs=2) as bpool,
            tc.tile_pool(name="sb", bufs=3) as pool,
            tc.tile_pool(name="epi", bufs=2) as epool,
            tc.tile_pool(name="psq", bufs=2, space="PSUM") as psq,
            tc.tile_pool(name="psacc", bufs=1, space="PSUM") as psa,
            tc.tile_pool(name="psepi", bufs=1, space="PSUM") as pse,
        ):
            wev = cpool.tile([EAW, OUT], bf)
            nc.sync.dma_start(wev[:], Wevd[:])
            ident = cpool.tile([BLKN, BLKN], bf)
            nc.sync.dma_start(ident[:], identd[:])

            for b in range(BLKS):
                qqe_b = bpool.tile([BLKN, HW2], bf, tag="qqe_b")
                nc.sync.dma_start(qqe_b[:], qqed[b * BLKN : (b + 1) * BLKN, :])
                s2b = bpool.tile([BLKN, OUT], bf, tag="s2b")
                nc.sync.dma_start(s2b[:], s2d[b * BLKN : (b + 1) * BLKN, :])
                Sb = bpool.tile([T, CPB * BLKN], bf, tag="Sb")
                nc.sync.dma_start(Sb[:], Sd[b])
                STb = bpool.tile([BLKN, CPB * T], bf, tag="STb")
                nc.sync.dma_start(STb[:], STd[b])
                Rb = bpool.tile([T, CPB * R2W], bf, tag="Rb")
                nc.sync.dma_start(Rb[:], Rd[b])

                psX = psa.tile([BLKN, HW2], f32, tag="psX")

                for i in range(CPB):
                    Rc = Rb[:, i * R2W : (i + 1) * R2W]
                    s_ = Sb[:, i * BLKN : (i + 1) * BLKN]
                    st_ = STb[:, i * T : (i + 1) * T]

                    qt = psq.tile([T, HW2], f32, tag="qt")
                    nc.tensor.matmul(qt[:], st_, qqe_b[:], start=True, stop=True)

                    prod = pool.tile([T, HW2], bf, tag="prod")
                    nc.vector.tensor_tensor(
                        prod[:], qt[:], Rc[:, :HW2], ALU.mult
                    )
                    alpha = pool.tile([T, 1], f32, tag="alpha")
                    nc.vector.tensor_reduce(
                        alpha[:], prod[:], mybir.AxisListType.X, ALU.add
                    )
                    ex = pool.tile([T, 1], f32, tag="ex")
                    nc.scalar.activation(ex[:], alpha[:], AF.Exp, scale=ISQ2)

                    EXV = pool.tile([T, HW2], bf, tag="EXV")
                    nc.scalar.activation(
                        EXV[:], Rc[:, HW2:R2W], AF.Copy, scale=ex[:]
                    )
                    nc.tensor.matmul(
                        psX[:], s_, EXV[:], start=(i == 0), stop=(i == CPB - 1)
                    )

                # ---- block epilogue ----
                eEb = epool.tile([BLKN, EAW], bf, tag="eEb")
                nc.scalar.activation(eEb[:], psX[:, OUT:HW2], AF.Copy)
                den = epool.tile([BLKN, 1], f32, tag="den")
                nc.vector.tensor_scalar_max(
                    den[:], psX[:, HW2 - 1 : HW2], DENOM_EPS
                )
                rcp = epool.tile([BLKN, 1], f32, tag="rcp")
                nc.vector.reciprocal(rcp[:], den[:])

                pst = pse.tile([EAW, BLKN], bf, tag="pst")
                nc.tensor.transpose(pst[:], eEb[:], ident[:])
                eET = epool.tile([EAW, BLKN], bf, tag="eET")
                nc.vector.tensor_scalar_add(eET[:], pst[:], 0.0)
                nc.tensor.matmul(
                    psX[:, :OUT], eET[:], wev[:],
                    start=False, stop=True, skip_group_check=True,
                )

                zt = epool.tile([BLKN, OUT], f32, tag="zt")
                nc.vector.scalar_tensor_tensor(
                    zt[:], psX[:, :OUT], rcp[:], s2b[:], ALU.mult, ALU.add
                )
                nc.sync.dma_start(zd[b * BLKN : (b + 1) * BLKN, :], zt[:])
    return nc


# ----------------------------------------------------------------------------
# Kernel entry point
# ----------------------------------------------------------------------------
PROFILE = False
LAST_EXEC_NS = None
LAST_TRACES = None


def kernel(**inputs):
    global LAST_EXEC_NS, LAST_TRACES
    _install_shim()
    import ml_dtypes

    from concourse import bass_utils

    bf = ml_dtypes.bfloat16

    def _run(nc, in_maps):
        r = bass_utils.run_bass_kernel_spmd(
            nc, in_maps, core_ids=list(range(NCORES)), trace=PROFILE
        )
        if PROFILE:
            _exec_ns.append(r.exec_time_ns)
            _traces.append(r.instructions_and_trace)
        return r

    _exec_ns, _traces = [], []

    x = np.asarray(inputs["x"], dtype=np.float32)
    ei = np.asarray(inputs["ei"])
    ea = np.asarray(inputs["ea"], dtype=np.float32)
    W = {k: np.asarray(v, dtype=np.float32) for k, v in inputs.items()
         if k not in ("x", "ei", "ea")}

    plan = _make_plan(ei)
    S_p, ST_p = _build_S_packed(plan)

    # per-edge [ea | 1] rows (zeros on padding slots)
    eid = plan.edge_id  # [cores, ELOC]
    valid = eid >= 0
    eap = np.zeros((NCORES, ELOC, EAW), dtype=np.float32)
    eap[valid, :EDGE_DIM] = ea[eid[valid]]
    eap[valid, EDGE_DIM] = 1.0
    eap = eap.astype(bf)

    # node features in slot order
    x_slots = np.zeros((NTOT, IN_DIM), dtype=np.float32)
    x_slots[plan.node_gslot] = x
    xT_all = np.ascontiguousarray(x_slots.T).astype(bf)  # [128, NTOT]

    # ------------- weight folding (host, f32) -------------
    Wq1, bq1 = W["Wq1"], W["bq1"]
    Wk1, bk1 = W["Wk1"], W["bk1"]
    Wv1, bv1 = W["Wv1"], W["bv1"]
    We1, Ws1, bs1 = W["We1"], W["Ws1"], W["bs1"]

    Wn = np.zeros((IN_DIM, NF0), dtype=np.float32)
    bn = np.zeros(NF0, dtype=np.float32)
    Wn[:, 0:F1] = Wk1.T          # k (no bias; bk rides qe' ones col)
    Wn[:, F1 : 2 * F1] = Wv1.T   # v (no bias; bv rides Wev ones row)
    for h in range(H1):
        Wq_h = Wq1[h * HID : (h + 1) * HID]       # [128, 128]
        bq_h = bq1[h * HID : (h + 1) * HID]
        We_h = We1[h * HID : (h + 1) * HID]       # [128, 32]
        bk_h = bk1[h * HID : (h + 1) * HID]
        M_h = np.concatenate([We_h, bk_h[:, None]], axis=1)  # [128, 33]
        c0 = 2 * F1 + h * HW1
        Wn[:, c0 : c0 + HID] = Wq_h.T
        Wn[:, c0 + HID : c0 + HW1] = Wq_h.T @ M_h
        bn[c0 : c0 + HID] = bq_h
        bn[c0 + HID : c0 + HW1] = bq_h @ M_h
    s0 = 2 * F1 + QQE1
    Wn[:, s0 : s0 + F1] = Ws1.T
    bn[s0 : s0 + F1] = bs1
    Wn_bf = Wn.astype(bf)
    bT = np.ascontiguousarray(bn.reshape(NCH0, 128).T)  # [128, 18] f32

    # Wevp: [66, 512]; head-pair block-diagonal [We_h^T; bv_h] expansions
    Wevp = np.zeros((2 * EAW, 2 * 256), dtype=np.float32)
    for h in range(H1):
        p, q = divmod(h, 2)
        r0, c0 = q * EAW, p * 256 + q * HID
        Wevp[r0 : r0 + EDGE_DIM, c0 : c0 + HID] = We1[h * HID : (h + 1) * HID].T
        Wevp[r0 + EDGE_DIM, c0 : c0 + HID] = bv1[h * HID : (h + 1) * HID]
    Wevp_bf = Wevp.astype(bf)

    # layer-2 node GEMM weights: rows [Wk2|Wv2|Wq2|Wqe2'|Ws2] of [O2W, 512]
    Wq2, bq2 = W["Wq2"], W["bq2"]
    Wk2, bk2 = W["Wk2"], W["bk2"]
    Wv2, bv2 = W["Wv2"], W["bv2"]
    We2, Ws2, bs2 = W["We2"], W["Ws2"], W["bs2"]
    M2 = np.concatenate([We2, bk2[:, None]], axis=1)  # [64, 33]
    W2cat = np.concatenate(
        [Wk2, Wv2, Wq2, M2.T @ Wq2, Ws2], axis=0
    )  # [289, 512]
    b2row = np.concatenate(
        [np.zeros(OUT), np.zeros(OUT), bq2, bq2 @ M2, bs2]
    ).astype(np.float32)  # [289]
    W2all = np.zeros((128, 4 * O2W), dtype=np.float32)
    for fb in range(4):
        W2all[:, fb * O2W : (fb + 1) * O2W] = W2cat[:, fb * 128 : (fb + 1) * 128].T
    W2all_bf = W2all.astype(bf)
    b2pad = np.zeros(384, dtype=np.float32)
    b2pad[:O2W] = b2row
    b2T = np.ascontiguousarray(b2pad.reshape(3, 128).T)  # [128, 3] f32

    Wev2 = np.concatenate([We2.T, bv2[None, :]], axis=0).astype(bf)  # [33, 64]

    ident = np.eye(BLKN, dtype=np.float32).astype(bf)
    ones = np.ones((1, BLKN), dtype=np.float32).astype(bf)

    # ---------------- launch 0: node phase ----------------
    if "n1" not in _built:
        _built["n1"] = _build_node1()
    in_maps0 = []
    for c in range(NCORES):
        in_maps0.append({
            "xT": np.ascontiguousarray(xT_all[:, c * NLOC : (c + 1) * NLOC]),
            "Wn": Wn_bf,
            "bT": bT,
        })
    r0 = _run(_built["n1"], in_maps0)
    nodeT_all = np.concatenate(
        [r0.results[c]["nodeT"] for c in range(NCORES)], axis=1
    )  # [NF0, NTOT] bf16
    rows = np.ascontiguousarray(nodeT_all.T)  # [NTOT, NF0]

    # host gather: per-edge R rows [[k_h|ea']x4 | [v_h|ea']x4]
    src_rows = rows[plan.edge_src_gslot.reshape(-1)]  # [cores*ELOC, NF0]
    kv = src_rows[:, : 2 * F1].reshape(-1, 2, H1, HID)
    R1 = np.zeros((NCORES * ELOC, 2, H1, HW1), dtype=bf)
    R1[:, :, :, :HID] = kv
    R1[:, :, :, HID:] = eap.reshape(-1, EAW)[:, None, None, :]
    R1p = _pack_rows_blocks(R1.reshape(NCORES, ELOC, R1W), R1W)

    qqe_all = rows[:, 2 * F1 : 2 * F1 + QQE1]  # [NTOT, 644]
    s1_all = rows[:, s0 : s0 + F1]

    # ---------------- launch 1: edge phase 1 -> h ----------------
    if "e1" not in _built:
        _built["e1"] = _build_edge1()
    in_maps1 = []
    for c in range(NCORES):
        in_maps1.append({
            "qqe": np.ascontiguousarray(qqe_all[c * NLOC : (c + 1) * NLOC]),
            "s1": np.ascontiguousarray(s1_all[c * NLOC : (c + 1) * NLOC]),
            "R": R1p[c],
            "S": S_p[c], "ST": ST_p[c],
            "Wevp": Wevp_bf,
            "ident": ident,
        })
    r1 = _run(_built["e1"], in_maps1)
    hT_all = np.ascontiguousarray(
        np.concatenate([r1.results[c]["h"] for c in range(NCORES)], axis=0).T
    )  # [F1, NTOT] bf16

    # ---------------- launch 1b: layer-2 node GEMM ----------------
    if "g2" not in _built:
        _built["g2"] = _build_gemm2()
    in_maps1b = []
    for c in range(NCORES):
        in_maps1b.append({
            "hT": np.ascontiguousarray(hT_all[:, c * NLOC : (c + 1) * NLOC]),
            "W2a": W2all_bf,
            "b2T": b2T,
        })
    r1b = _run(_built["g2"], in_maps1b)
    out2_all = np.ascontiguousarray(
        np.concatenate([r1b.results[c]["out2T"] for c in range(NCORES)], axis=1).T
    )  # [NTOT, 289] bf16

    # host gather for layer 2
    src2 = out2_all[plan.edge_src_gslot.reshape(-1)]  # [cores*ELOC, 289]
    R2 = np.zeros((NCORES * ELOC, R2W), dtype=bf)
    R2[:, :OUT] = src2[:, :OUT]                 # k2
    R2[:, OUT:HW2] = eap.reshape(-1, EAW)       # ea'
    R2[:, HW2 : HW2 + OUT] = src2[:, OUT : 2 * OUT]  # v2
    R2[:, HW2 + OUT :] = eap.reshape(-1, EAW)
    R2p = _pack_rows_blocks(R2.reshape(NCORES, ELOC, R2W), R2W)

    qqe2_all = out2_all[:, 2 * OUT : 2 * OUT + HW2]
    s2_all = out2_all[:, 2 * OUT + HW2 :]

    # ---------------- launch 2: edge phase 2 ----------------
    if "e2" not in _built:
        _built["e2"] = _build_edge2()
    in_maps2 = []
    for c in range(NCORES):
        in_maps2.append({
            "qqe2": np.ascontiguousarray(qqe2_all[c * NLOC : (c + 1) * NLOC]),
            "s2": np.ascontiguousarray(s2_all[c * NLOC : (c + 1) * NLOC]),
            "R2": R2p[c],
            "S": S_p[c], "ST": ST_p[c],
            "Wev2": Wev2, "ident": ident,
        })
    r2 = _run(_built["e2"], in_maps2)
    z_all = np.concatenate([r2.results[c]["z"] for c in range(NCORES)], axis=0)

    z = z_all[plan.node_gslot]
    if PROFILE:
        LAST_EXEC_NS = sum(int(t) for t in _exec_ns if t) if all(_exec_ns) else None
        LAST_TRACES = _traces
    return z.astype(np.float32)


# revision 21
# speedup vs baseline: 1.4237x; 1.0334x over previous
"""TransformerConv 2-layer GNN encoder on 8 Trainium2 NeuronCores.

Strategy (dst-sharded graph parallelism, v2):
  - Nodes assigned to 8 cores x 20 blocks x 128 slots (degree-balanced FFD);
    each block has <= 1024 incoming edges -> 8 chunks of 128 edges.
  - Launch 0 (node1): feature-major GEMM nodeT = Wn^T @ x per core producing
    k|v|q|qe'|s rows, biases folded in via per-partition ACT/DVE bias.
    qe' = q @ [We_h | bk_h] folds the edge-attr term and k-bias of the
    attention logit into a 33-wide per-node vector (alpha = q.k + qe'.ea').
  - Host gathers per-edge source rows [k_h|ea']x4 | [v_h|ea']x4 (pure data
    movement between launches).
  - Launch 1 (edge1): per chunk: PE gathers [q|qe'] rows via one-hot ST
    matmuls, DVE tensor_tensor_reduce computes alpha per head straight from
    PSUM, ACT exponentiates and scales [v_h|ea'] by ex, PE scatters via
    one-hot S into numerator + 33-wide [ex*ea'|ex] accumulators. Block
    epilogue expands the ea'-accumulator through [We;bv] (the ones column
    doubles as softmax denominator and v-bias), normalizes, adds skip, relu,
    and runs the fused layer-2 node GEMM producing k2|v2|q2|qe2'|s2.
  - Launch 2 (edge2): same edge pipeline at width 64/33 -> z.
"""

import sys

sys.path.insert(0, "/opt/trn_rl_repo")

import json

import numpy as np

# ----------------------------------------------------------------------------
# Problem constants (hardcoded per contract)
# ----------------------------------------------------------------------------
N, E, IN_DIM, EDGE_DIM, HID, OUT = 20000, 160000, 128, 32, 128, 64
H1 = 4
F1 = H1 * HID  # 512
NCORES = 8
BLKS = 20          # dst blocks per core
BLKN = 128         # nodes per block
NLOC = BLKS * BLKN  # 2560 nodes per core
NTOT = NCORES * NLOC  # 20480 slots
CPB = 8            # chunks per block
T = 128            # edges per chunk
ECHUNKS = BLKS * CPB  # 160 chunks per core
ELOC = ECHUNKS * T    # 20480 edge slots per core

EAW = EDGE_DIM + 1          # 33: [ea | 1]
HW1 = HID + EAW             # 161: per-head [k|ea'] / [q|qe'] width
R1W = 2 * H1 * HW1          # 1288 gathered row width, layer 1
QQE1 = H1 * HW1             # 644
NF0 = 2304                  # node1 output rows (2180 used, padded to 18*128)
NCH0 = NF0 // 128           # 18

HW2 = OUT + EAW             # 97
R2W = 2 * HW2               # 194
O2W = 4 * OUT + EAW         # 289: k2|v2|q2|qe2'|s2

ISQ1 = 1.0 / np.sqrt(np.float32(HID))
ISQ2 = 1.0 / np.sqrt(np.float32(OUT))
DENOM_EPS = 1e-30

# ----------------------------------------------------------------------------
# Walrus single-wait shim + NTFF profiling hook (inlined; must be
# self-contained).
# ----------------------------------------------------------------------------
_shim_installed = False


def _split_waits_in_bir(bir_bytes: bytes) -> bytes:
    d = json.loads(bir_bytes)
    for fn in d.get("functions", []):
        for blk in fn.get("blocks", []):
            new_insts = []
            for ins in blk.get("instructions", []):
                si = ins.get("sync_info") or {}
                waits = si.get("on_wait") or []
                if len(waits) > 1:
                    for k, w in enumerate(waits[:-1]):
                        ev = {
                            "name": f"{ins['name']}_wsplit{k}",
                            "opcode": "EventSemaphore",
                            "engine": ins["engine"],
                            "ins": [],
                            "outs": [],
                            "sync_info": {"on_wait": [w], "on_update": []},
                        }
                        if "debug" in ins:
                            ev["debug"] = ins["debug"]
                        new_insts.append(ev)
                    si["on_wait"] = [waits[-1]]
                new_insts.append(ins)
            blk["instructions"] = new_insts
    return json.dumps(d).encode()


def _install_shim():
    global _shim_installed
    if _shim_installed:
        return
    import concourse.bass2jax as bass2jax
    import concourse.bass_utils as bass_utils

    orig = bass_utils.compile_bir_kernel

    def wrapped(bir_json, tmpdir, neff_name="file.neff"):
        if isinstance(bir_json, str):
            bir_json = bir_json.encode()
        return orig(_split_waits_in_bir(bir_json), tmpdir, neff_name=neff_name)

    bass_utils.compile_bir_kernel = wrapped
    bass2jax.compile_bir_kernel = wrapped

    import types

    try:
        from antenv import axon_hooks  # noqa: F401
    except ImportError:
        import antenv

        mod = types.ModuleType("antenv.axon_hooks")
        _state = {"hook": None}
        mod.set_axon_ntff_profile_hook = lambda h: _state.__setitem__("hook", h)
        mod.get_axon_ntff_profile_hook = lambda: _state["hook"]
        sys.modules["antenv.axon_hooks"] = mod
        antenv.axon_hooks = mod
        try:
            from trn_agent_boot.trn_boot import _ntff_profile_via_ctypes

            hook = _ntff_profile_via_ctypes("/opt/axon/libaxon_pjrt.so")
            if hook is not None:
                mod.set_axon_ntff_profile_hook(hook)
        except Exception:
            pass
    _shim_installed = True


# ----------------------------------------------------------------------------
# Host-side graph planning
# ----------------------------------------------------------------------------
class _Plan:
    pass


def _make_plan(ei: np.ndarray) -> _Plan:
    """Assign nodes to (core, block, slot); build per-core edge schedule."""
    src = np.asarray(ei[0], dtype=np.int64)
    dst = np.asarray(ei[1], dtype=np.int64)
    deg = np.bincount(dst, minlength=N)  # in-degree

    nbins = NCORES * BLKS  # 160
    cap_e = CPB * T  # 1024 edges per bin
    cap_n = BLKN  # 128 nodes per bin

    order = np.argsort(-deg, kind="stable")
    bin_e = np.zeros(nbins, dtype=np.int64)
    bin_n = np.zeros(nbins, dtype=np.int64)
    node_bin = np.empty(N, dtype=np.int64)
    start = 0
    for nd in order:
        d = deg[nd]
        placed = False
        for k in range(nbins):
            b = (start + k) % nbins
            if bin_e[b] + d <= cap_e and bin_n[b] < cap_n:
                node_bin[nd] = b
                bin_e[b] += d
                bin_n[b] += 1
                start = (b + 1) % nbins
                placed = True
                break
        if not placed:
            raise RuntimeError("bin packing failed")

    node_slot = np.empty(N, dtype=np.int64)
    fill = np.zeros(nbins, dtype=np.int64)
    for nd in range(N):
        b = node_bin[nd]
        node_slot[nd] = fill[b]
        fill[b] += 1

    node_gslot = node_bin * BLKN + node_slot  # in [0, NTOT)

    edge_bin = node_bin[dst]
    eorder = np.argsort(edge_bin, kind="stable")
    sorted_bins = edge_bin[eorder]
    bin_starts = np.searchsorted(sorted_bins, np.arange(nbins))
    bin_ends = np.searchsorted(sorted_bins, np.arange(nbins), side="right")

    edge_src_gslot = np.zeros((NCORES, ELOC), dtype=np.int64)
    edge_id = np.full((NCORES, ELOC), -1, dtype=np.int64)
    edge_dslot = np.full((NCORES, ELOC), -1, dtype=np.int64)
    for b in range(nbins):
        core = b // BLKS
        blk = b % BLKS
        s, e = bin_starts[b], bin_ends[b]
        eids = eorder[s:e]
        ne = len(eids)
        base = blk * cap_e
        edge_id[core, base : base + ne] = eids
        edge_src_gslot[core, base : base + ne] = node_gslot[src[eids]]
        edge_dslot[core, base : base + ne] = node_slot[dst[eids]]

    p = _Plan()
    p.node_gslot = node_gslot
    p.edge_src_gslot = edge_src_gslot
    p.edge_id = edge_id
    p.edge_dslot = edge_dslot
    return p


def _build_S_packed(plan):
    """Per-block packed one-hot matrices, bf16."""
    import ml_dtypes

    S = np.zeros((NCORES, ECHUNKS, T, BLKN), dtype=np.float32)
    dslot = plan.edge_dslot.reshape(NCORES, ECHUNKS, T)
    c_idx, ch_idx, t_idx = np.nonzero(dslot >= 0)
    S[c_idx, ch_idx, t_idx, dslot[c_idx, ch_idx, t_idx]] = 1.0
    bf = ml_dtypes.bfloat16
    S_p = np.ascontiguousarray(
        S.reshape(NCORES, BLKS, CPB, T, BLKN).transpose(0, 1, 3, 2, 4)
        .reshape(NCORES, BLKS, T, CPB * BLKN)
    ).astype(bf)
    ST_p = np.ascontiguousarray(
        S.reshape(NCORES, BLKS, CPB, T, BLKN).transpose(0, 1, 4, 2, 3)
        .reshape(NCORES, BLKS, BLKN, CPB * T)
    ).astype(bf)
    return S_p, ST_p


def _pack_rows_blocks(rows, width):
    """[cores, ELOC, width] -> [cores, BLKS, T, CPB*width] block-major."""
    return np.ascontiguousarray(
        rows.reshape(NCORES, BLKS, CPB, T, width).transpose(0, 1, 3, 2, 4)
        .reshape(NCORES, BLKS, T, CPB * width)
    )


# ----------------------------------------------------------------------------
# Bass kernel builders
# ----------------------------------------------------------------------------
_built = {}


def _get_nc():
    import concourse.bass as bass

    return bass.Bass(target_bir_lowering=False, trn_type="TRN2")


def _build_node1():
    """Feature-major node GEMM: nodeT[f, n] = sum_d Wn[d, f] x[n, d] + bias[f]."""
    import concourse.mybir as mybir
    from concourse.tile import TileContext

    dt = mybir.dt
    nc = _get_nc()
    f32, bf = dt.float32, dt.bfloat16
    AF = mybir.ActivationFunctionType

    xT = nc.dram_tensor("xT", [IN_DIM, NLOC], bf, kind="ExternalInput")
    Wn = nc.dram_tensor("Wn", [IN_DIM, NF0], bf, kind="ExternalInput")
    bT = nc.dram_tensor("bT", [128, NCH0], f32, kind="ExternalInput")
    nodeT = nc.dram_tensor("nodeT", [NF0, NLOC], bf, kind="ExternalOutput")

    with TileContext(nc) as tc:
        with (
            tc.tile_pool(name="const", bufs=1) as cpool,
            tc.tile_pool(name="ob", bufs=2) as opool,
            tc.tile_pool(name="ps", bufs=6, space="PSUM") as psp,
        ):
            w = cpool.tile([IN_DIM, NF0], bf)
            nc.sync.dma_start(w[:], Wn[:])
            bt = cpool.tile([128, NCH0], f32)
            nc.sync.dma_start(bt[:], bT[:])
            xfull = cpool.tile([IN_DIM, NLOC], bf)
            nc.sync.dma_start(xfull[:], xT[:])
            for fc in range(NCH0):
                wslice = w[:, fc * 128 : (fc + 1) * 128]
                ob = opool.tile([128, NLOC], bf, tag="ob")
                for g in range(NLOC // 512):
                    ps = psp.tile([128, 512], f32, tag="ps")
                    nc.tensor.matmul(
                        ps[:], wslice, xfull[:, g * 512 : (g + 1) * 512],
                        start=True, stop=True,
                    )
                    osl = ob[:, g * 512 : (g + 1) * 512]
                    if g % 2 == 1:
                        nc.vector.tensor_scalar_add(osl, ps[:], bt[:, fc : fc + 1])
                    else:
                        nc.scalar.activation(
                            osl, ps[:], AF.Identity, bias=bt[:, fc : fc + 1]
                        )
                nc.sync.dma_start(nodeT[fc * 128 : (fc + 1) * 128, :], ob[:])
    return nc


def _build_edge1():
    """Layer-1 edge phase -> h (relu'd layer-1 output), node-major."""
    import concourse.mybir as mybir
    from concourse.tile import TileContext

    dt = mybir.dt
    nc = _get_nc()
    f32, bf = dt.float32, dt.bfloat16
    AF = mybir.ActivationFunctionType
    ALU = mybir.AluOpType

    qqed = nc.dram_tensor("qqe", [NLOC, QQE1], bf, kind="ExternalInput")
    s1d = nc.dram_tensor("s1", [NLOC, F1], bf, kind="ExternalInput")
    Rd = nc.dram_tensor("R", [BLKS, T, CPB * R1W], bf, kind="ExternalInput")
    Sd = nc.dram_tensor("S", [BLKS, T, CPB * BLKN], bf, kind="ExternalInput")
    STd = nc.dram_tensor("ST", [BLKS, BLKN, CPB * T], bf, kind="ExternalInput")
    Wevd = nc.dram_tensor("Wevp", [2 * EAW, 2 * 256], bf, kind="ExternalInput")
    identd = nc.dram_tensor("ident", [BLKN, BLKN], bf, kind="ExternalInput")
    hd = nc.dram_tensor("h", [NLOC, F1], bf, kind="ExternalOutput")

    with TileContext(nc) as tc:
        with (
            tc.tile_pool(name="const", bufs=1) as cpool,
            tc.tile_pool(name="blk", bufs=3) as bpool,
            tc.tile_pool(name="sb", bufs=4) as pool,
            tc.tile_pool(name="epi", bufs=2) as epool,
            tc.tile_pool(name="psq", bufs=2, space="PSUM") as psq,
            tc.tile_pool(name="psacc", bufs=1, space="PSUM") as psa,
            tc.tile_pool(name="psepi", bufs=2, space="PSUM") as pse,
        ):
            wevp = cpool.tile([2 * EAW, 2 * 256], bf)
            nc.sync.dma_start(wevp[:], Wevd[:])
            ident = cpool.tile([BLKN, BLKN], bf)
            nc.sync.dma_start(ident[:], identd[:])

            for b in range(BLKS):
                qqe_b = bpool.tile([BLKN, QQE1], bf, tag="qqe_b")
                nc.sync.dma_start(qqe_b[:], qqed[b * BLKN : (b + 1) * BLKN, :])
                s1b = bpool.tile([BLKN, F1], bf, tag="s1b")
                nc.sync.dma_start(s1b[:], s1d[b * BLKN : (b + 1) * BLKN, :])
                Sb = bpool.tile([T, CPB * BLKN], bf, tag="Sb")
                nc.sync.dma_start(Sb[:], Sd[b])
                STb = bpool.tile([BLKN, CPB * T], bf, tag="STb")
                nc.sync.dma_start(STb[:], STd[b])
                Rb = bpool.tile([T, CPB * R1W], bf, tag="Rb")
                nc.sync.dma_start(Rb[:], Rd[b])

                psN = psa.tile([BLKN, F1], f32, tag="psN")
                psE = psa.tile([BLKN, H1 * EAW], f32, tag="psE")

                for i in range(CPB):
                    Rc = Rb[:, i * R1W : (i + 1) * R1W]
                    s_ = Sb[:, i * BLKN : (i + 1) * BLKN]
                    st_ = STb[:, i * T : (i + 1) * T]

                    qtA = psq.tile([T, 3 * HW1], f32, tag="qtA")
                    nc.tensor.matmul(
                        qtA[:], st_, qqe_b[:, : 3 * HW1], start=True, stop=True
                    )
                    qtB = psq.tile([T, HW1], f32, tag="qtB")
                    nc.tensor.matmul(
                        qtB[:], st_, qqe_b[:, 3 * HW1 : QQE1], start=True, stop=True
                    )

                    # copy gathered [q|qe'] to SBUF bf16 (ACT + DVE split)
                    qtqeb = pool.tile([T, QQE1], bf, tag="qtqeb")
                    nc.scalar.activation(qtqeb[:, : 3 * HW1], qtA[:], AF.Copy)
                    nc.vector.tensor_scalar_add(qtqeb[:, 3 * HW1 :], qtB[:], 0.0)

                    prod = pool.tile([T, QQE1], bf, tag="prod")
                    nc.vector.tensor_tensor(
                        prod[:], qtqeb[:], Rc[:, :QQE1], ALU.mult
                    )
                    alpha = pool.tile([T, H1], f32, tag="alpha")
                    nc.vector.tensor_reduce(
                        alpha[:],
                        prod[:].rearrange("p (h w) -> p h w", h=H1),
                        mybir.AxisListType.X,
                        ALU.add,
                    )
                    ex4 = pool.tile([T, H1], f32, tag="ex4")
                    nc.scalar.activation(ex4[:], alpha[:], AF.Exp, scale=ISQ1)

                    EXV = pool.tile([T, QQE1], bf, tag="EXV")
                    for h in range(H1):
                        vsl = Rc[:, QQE1 + h * HW1 : QQE1 + (h + 1) * HW1]
                        esl = EXV[:, h * HW1 : (h + 1) * HW1]
                        exh = ex4[:, h : h + 1]
                        if h == 1:
                            nc.vector.tensor_scalar_mul(esl, vsl, exh)
                        else:
                            nc.scalar.activation(esl, vsl, AF.Copy, scale=exh)
                    ev3 = EXV[:].rearrange("p (h w) -> p h w", h=H1)
                    nc.tensor.matmul(
                        psN[:], s_, ev3[:, :, :HID],
                        start=(i == 0), stop=False,
                    )
                    nc.tensor.matmul(
                        psE[:], s_, ev3[:, :, HID:],
                        start=(i == 0), stop=(i == CPB - 1),
                    )

                # ---- block epilogue ----
                eEb = epool.tile([BLKN, H1 * EAW], bf, tag="eEb")
                nc.scalar.activation(eEb[:], psE[:], AF.Copy)
                den = epool.tile([BLKN, H1], f32, tag="den")
                nc.vector.tensor_scalar_max(
                    den[:].rearrange("p (h o) -> p h o", o=1),
                    psE[:].rearrange("p (h w) -> p h w", h=H1)[:, :, EDGE_DIM : EDGE_DIM + 1],
                    DENOM_EPS,
                )
                rcp = epool.tile([BLKN, H1], f32, tag="rcp")
                nc.vector.reciprocal(rcp[:], den[:])

                for p in range(2):
                    pst = pse.tile([2 * EAW, BLKN], bf, tag="pst")
                    nc.tensor.transpose(
                        pst[:], eEb[:, p * 2 * EAW : (p + 1) * 2 * EAW], ident[:]
                    )
                    eET = epool.tile([2 * EAW, BLKN], bf, tag="eET")
                    nc.vector.tensor_scalar_add(eET[:], pst[:], 0.0)
                    nc.tensor.matmul(
                        psN[:, p * 256 : (p + 1) * 256],
                        eET[:],
                        wevp[:, p * 256 : (p + 1) * 256],
                        start=False,
                        stop=True,
                        skip_group_check=True,
                    )

                hb = epool.tile([BLKN, F1], bf, tag="hb")
                for h in range(H1):
                    nc.vector.scalar_tensor_tensor(
                        hb[:, h * HID : (h + 1) * HID],
                        psN[:, h * HID : (h + 1) * HID],
                        rcp[:, h : h + 1],
                        s1b[:, h * HID : (h + 1) * HID],
                        ALU.mult,
                        ALU.add,
                    )
                hr = epool.tile([BLKN, F1], bf, tag="hr")
                nc.vector.tensor_scalar_max(hr[:], hb[:], 0.0)
                nc.sync.dma_start(hd[b * BLKN : (b + 1) * BLKN, :], hr[:])
    return nc


def _build_gemm2():
    """Feature-major layer-2 node GEMM: out2T = W2cat @ h^T + b2."""
    import concourse.mybir as mybir
    from concourse.tile import TileContext

    dt = mybir.dt
    nc = _get_nc()
    f32, bf = dt.float32, dt.bfloat16
    AF = mybir.ActivationFunctionType

    hTd = nc.dram_tensor("hT", [F1, NLOC], bf, kind="ExternalInput")
    W2d = nc.dram_tensor("W2a", [128, 4 * O2W], bf, kind="ExternalInput")
    b2Td = nc.dram_tensor("b2T", [128, 3], f32, kind="ExternalInput")
    out2T = nc.dram_tensor("out2T", [O2W, NLOC], bf, kind="ExternalOutput")

    FCS = [(0, 128), (128, 256), (256, O2W)]
    with TileContext(nc) as tc:
        with (
            tc.tile_pool(name="const", bufs=1) as cpool,
            tc.tile_pool(name="ob", bufs=2) as opool,
            tc.tile_pool(name="ps", bufs=6, space="PSUM") as psp,
        ):
            w2a = cpool.tile([128, 4 * O2W], bf)
            nc.sync.dma_start(w2a[:], W2d[:])
            b2t = cpool.tile([128, 3], f32)
            nc.sync.dma_start(b2t[:], b2Td[:])
            hts = []
            for hk in range(4):
                ht = cpool.tile([128, NLOC], bf, name=f"ht{hk}")
                nc.sync.dma_start(ht[:], hTd[hk * 128 : (hk + 1) * 128, :])
                hts.append(ht)
            for ci, (f0, f1) in enumerate(FCS):
                fw = f1 - f0
                ob = opool.tile([128, NLOC], bf, tag="ob")
                for g in range(NLOC // 512):
                    ps = psp.tile([128, 512], f32, tag="ps")
                    for hk in range(4):
                        nc.tensor.matmul(
                            ps[:fw, :],
                            w2a[:, hk * O2W + f0 : hk * O2W + f1],
                            hts[hk][:, g * 512 : (g + 1) * 512],
                            start=(hk == 0), stop=(hk == 3),
                        )
                    osl = ob[:fw, g * 512 : (g + 1) * 512]
                    if g % 3 == 2:
                        nc.vector.tensor_scalar_add(
                            osl, ps[:fw, :], b2t[:fw, ci : ci + 1]
                        )
                    else:
                        nc.scalar.activation(
                            osl, ps[:fw, :], AF.Identity,
                            bias=b2t[:fw, ci : ci + 1],
                        )
                nc.sync.dma_start(out2T[f0:f1, :], ob[:fw, :])
    return nc


def _build_edge2():
    """Layer-2 edge phase -> z."""
    import concourse.mybir as mybir
    from concourse.tile import TileContext

    dt = mybir.dt
    nc = _get_nc()
    f32, bf = dt.float32, dt.bfloat16
    AF = mybir.ActivationFunctionType
    ALU = mybir.AluOpType

    qqed = nc.dram_tensor("qqe2", [NLOC, HW2], bf, kind="ExternalInput")
    s2d = nc.dram_tensor("s2", [NLOC, OUT], bf, kind="ExternalInput")
    Rd = nc.dram_tensor("R2", [BLKS, T, CPB * R2W], bf, kind="ExternalInput")
    Sd = nc.dram_tensor("S", [BLKS, T, CPB * BLKN], bf, kind="ExternalInput")
    STd = nc.dram_tensor("ST", [BLKS, BLKN, CPB * T], bf, kind="ExternalInput")
    Wevd = nc.dram_tensor("Wev2", [EAW, OUT], bf, kind="ExternalInput")
    identd = nc.dram_tensor("ident", [BLKN, BLKN], bf, kind="ExternalInput")
    zd = nc.dram_tensor("z", [NLOC, OUT], f32, kind="ExternalOutput")

    with TileContext(nc) as tc:
        with (
            tc.tile_pool(name="const", bufs=1) as cpool,
            tc.tile_pool(name="blk", bufs=2) as bpool,
            tc.tile_pool(name="sb", bufs=3) as pool,
            tc.tile_pool(name="epi", bufs=2) as epool,
            tc.tile_pool(name="psq", bufs=2, space="PSUM") as psq,
            tc.tile_pool(name="psacc", bufs=1, space="PSUM") as psa,
            tc.tile_pool(name="psepi", bufs=1, space="PSUM") as pse,
        ):
            wev = cpool.tile([EAW, OUT], bf)
            nc.sync.dma_start(wev[:], Wevd[:])
            ident = cpool.tile([BLKN, BLKN], bf)
            nc.sync.dma_start(ident[:], identd[:])

            for b in range(BLKS):
                qqe_b = bpool.tile([BLKN, HW2], bf, tag="qqe_b")
                nc.sync.dma_start(qqe_b[:], qqed[b * BLKN : (b + 1) * BLKN, :])
                s2b = bpool.tile([BLKN, OUT], bf, tag="s2b")
                nc.sync.dma_start(s2b[:], s2d[b * BLKN : (b + 1) * BLKN, :])
                Sb = bpool.tile([T, CPB * BLKN], bf, tag="Sb")
                nc.sync.dma_start(Sb[:], Sd[b])
                STb = bpool.tile([BLKN, CPB * T], bf, tag="STb")
                nc.sync.dma_start(STb[:], STd[b])
                Rb = bpool.tile([T, CPB * R2W], bf, tag="Rb")
                nc.sync.dma_start(Rb[:], Rd[b])

                psX = psa.tile([BLKN, HW2], f32, tag="psX")

                for i in range(CPB):
                    Rc = Rb[:, i * R2W : (i + 1) * R2W]
                    s_ = Sb[:, i * BLKN : (i + 1) * BLKN]
                    st_ = STb[:, i * T : (i + 1) * T]

                    qt = psq.tile([T, HW2], f32, tag="qt")
                    nc.tensor.matmul(qt[:], st_, qqe_b[:], start=True, stop=True)

                    prod = pool.tile([T, HW2], bf, tag="prod")
                    nc.vector.tensor_tensor(
                        prod[:], qt[:], Rc[:, :HW2], ALU.mult
                    )
                    alpha = pool.tile([T, 1], f32, tag="alpha")
                    nc.vector.tensor_reduce(
                        alpha[:], prod[:], mybir.AxisListType.X, ALU.add
                    )
                    ex = pool.tile([T, 1], f32, tag="ex")
                    nc.scalar.activation(ex[:], alpha[:], AF.Exp, scale=ISQ2)

                    EXV = pool.tile([T, HW2], bf, tag="EXV")
                    nc.vector.tensor_scalar_mul(EXV[:], Rc[:, HW2:R2W], ex[:])
                    nc.tensor.matmul(
                        psX[:], s_, EXV[:], start=(i == 0), stop=(i == CPB - 1)
                    )

                # ---- block epilogue ----
                eEb = epool.tile([BLKN, EAW], bf, tag="eEb")
                nc.scalar.activation(eEb[:], psX[:, OUT:HW2], AF.Copy)
                den = epool.tile([BLKN, 1], f32, tag="den")
                nc.vector.tensor_scalar_max(
                    den[:], psX[:, HW2 - 1 : HW2], DENOM_EPS
                )
                rcp = epool.tile([BLKN, 1], f32, tag="rcp")
                nc.vector.reciprocal(rcp[:], den[:])

                pst = pse.tile([EAW, BLKN], bf, tag="pst")
                nc.tensor.transpose(pst[:], eEb[:], ident[:])
                eET = epool.tile([EAW, BLKN], bf, tag="eET")
                nc.vector.tensor_scalar_add(eET[:], pst[:], 0.0)
                nc.tensor.matmul(
                    psX[:, :OUT], eET[:], wev[:],
                    start=False, stop=True, skip_group_check=True,
                )

                zt = epool.tile([BLKN, OUT], f32, tag="zt")
                nc.vector.scalar_tensor_tensor(
                    zt[:], psX[:, :OUT], rcp[:], s2b[:], ALU.mult, ALU.add
                )
                nc.sync.dma_start(zd[b * BLKN : (b + 1) * BLKN, :], zt[:])
    return nc


# ----------------------------------------------------------------------------
# Kernel entry point
# ----------------------------------------------------------------------------
PROFILE = False
LAST_EXEC_NS = None
LAST_TRACES = None


def kernel(**inputs):
    global LAST_EXEC_NS, LAST_TRACES
    _install_shim()
    import ml_dtypes

    from concourse import bass_utils

    bf = ml_dtypes.bfloat16

    def _run(nc, in_maps):
        r = bass_utils.run_bass_kernel_spmd(
            nc, in_maps, core_ids=list(range(NCORES)), trace=PROFILE
        )
        if PROFILE:
            _exec_ns.append(r.exec_time_ns)
            _traces.append(r.instructions_and_trace)
        return r

    _exec_ns, _traces = [], []

    x = np.asarray(inputs["x"], dtype=np.float32)
    ei = np.asarray(inputs["ei"])
    ea = np.asarray(inputs["ea"], dtype=np.float32)
    W = {k: np.asarray(v, dtype=np.float32) for k, v in inputs.items()
         if k not in ("x", "ei", "ea")}

    plan = _make_plan(ei)
    S_p, ST_p = _build_S_packed(plan)

    # per-edge [ea | 1] rows (zeros on padding slots)
    eid = plan.edge_id  # [cores, ELOC]
    valid = eid >= 0
    eap = np.zeros((NCORES, ELOC, EAW), dtype=np.float32)
    eap[valid, :EDGE_DIM] = ea[eid[valid]]
    eap[valid, EDGE_DIM] = 1.0
    eap = eap.astype(bf)

    # node features in slot order
    x_slots = np.zeros((NTOT, IN_DIM), dtype=np.float32)
    x_slots[plan.node_gslot] = x
    xT_all = np.ascontiguousarray(x_slots.T).astype(bf)  # [128, NTOT]

    # ------------- weight folding (host, f32) -------------
    Wq1, bq1 = W["Wq1"], W["bq1"]
    Wk1, bk1 = W["Wk1"], W["bk1"]
    Wv1, bv1 = W["Wv1"], W["bv1"]
    We1, Ws1, bs1 = W["We1"], W["Ws1"], W["bs1"]

    Wn = np.zeros((IN_DIM, NF0), dtype=np.float32)
    bn = np.zeros(NF0, dtype=np.float32)
    Wn[:, 0:F1] = Wk1.T          # k (no bias; bk rides qe' ones col)
    Wn[:, F1 : 2 * F1] = Wv1.T   # v (no bias; bv rides Wev ones row)
    for h in range(H1):
        Wq_h = Wq1[h * HID : (h + 1) * HID]       # [128, 128]
        bq_h = bq1[h * HID : (h + 1) * HID]
        We_h = We1[h * HID : (h + 1) * HID]       # [128, 32]
        bk_h = bk1[h * HID : (h + 1) * HID]
        M_h = np.concatenate([We_h, bk_h[:, None]], axis=1)  # [128, 33]
        c0 = 2 * F1 + h * HW1
        Wn[:, c0 : c0 + HID] = Wq_h.T
        Wn[:, c0 + HID : c0 + HW1] = Wq_h.T @ M_h
        bn[c0 : c0 + HID] = bq_h
        bn[c0 + HID : c0 + HW1] = bq_h @ M_h
    s0 = 2 * F1 + QQE1
    Wn[:, s0 : s0 + F1] = Ws1.T
    bn[s0 : s0 + F1] = bs1
    Wn_bf = Wn.astype(bf)
    bT = np.ascontiguousarray(bn.reshape(NCH0, 128).T)  # [128, 18] f32

    # Wevp: [66, 512]; head-pair block-diagonal [We_h^T; bv_h] expansions
    Wevp = np.zeros((2 * EAW, 2 * 256), dtype=np.float32)
    for h in range(H1):
        p, q = divmod(h, 2)
        r0, c0 = q * EAW, p * 256 + q * HID
        Wevp[r0 : r0 + EDGE_DIM, c0 : c0 + HID] = We1[h * HID : (h + 1) * HID].T
        Wevp[r0 + EDGE_DIM, c0 : c0 + HID] = bv1[h * HID : (h + 1) * HID]
    Wevp_bf = Wevp.astype(bf)

    # layer-2 node GEMM weights: rows [Wk2|Wv2|Wq2|Wqe2'|Ws2] of [O2W, 512]
    Wq2, bq2 = W["Wq2"], W["bq2"]
    Wk2, bk2 = W["Wk2"], W["bk2"]
    Wv2, bv2 = W["Wv2"], W["bv2"]
    We2, Ws2, bs2 = W["We2"], W["Ws2"], W["bs2"]
    M2 = np.concatenate([We2, bk2[:, None]], axis=1)  # [64, 33]
    W2cat = np.concatenate(
        [Wk2, Wv2, Wq2, M2.T @ Wq2, Ws2], axis=0
    )  # [289, 512]
    b2row = np.concatenate(
        [np.zeros(OUT), np.zeros(OUT), bq2, bq2 @ M2, bs2]
    ).astype(np.float32)  # [289]
    W2all = np.zeros((128, 4 * O2W), dtype=np.float32)
    for fb in range(4):
        W2all[:, fb * O2W : (fb + 1) * O2W] = W2cat[:, fb * 128 : (fb + 1) * 128].T
    W2all_bf = W2all.astype(bf)
    b2pad = np.zeros(384, dtype=np.float32)
    b2pad[:O2W] = b2row
    b2T = np.ascontiguousarray(b2pad.reshape(3, 128).T)  # [128, 3] f32

    Wev2 = np.concatenate([We2.T, bv2[None, :]], axis=0).astype(bf)  # [33, 64]

    ident = np.eye(BLKN, dtype=np.float32).astype(bf)
    ones = np.ones((1, BLKN), dtype=np.float32).astype(bf)

    # ---------------- launch 0: node phase ----------------
    if "n1" not in _built:
        _built["n1"] = _build_node1()
    in_maps0 = []
    for c in range(NCORES):
        in_maps0.append({
            "xT": np.ascontiguousarray(xT_all[:, c * NLOC : (c + 1) * NLOC]),
            "Wn": Wn_bf,
            "bT": bT,
        })
    r0 = _run(_built["n1"], in_maps0)
    nodeT_all = np.concatenate(
        [r0.results[c]["nodeT"] for c in range(NCORES)], axis=1
    )  # [NF0, NTOT] bf16
    rows = np.ascontiguousarray(nodeT_all.T)  # [NTOT, NF0]

    # host gather: per-edge R rows [[k_h|ea']x4 | [v_h|ea']x4]
    src_rows = rows[plan.edge_src_gslot.reshape(-1)]  # [cores*ELOC, NF0]
    kv = src_rows[:, : 2 * F1].reshape(-1, 2, H1, HID)
    R1 = np.zeros((NCORES * ELOC, 2, H1, HW1), dtype=bf)
    R1[:, :, :, :HID] = kv
    R1[:, :, :, HID:] = eap.reshape(-1, EAW)[:, None, None, :]
    R1p = _pack_rows_blocks(R1.reshape(NCORES, ELOC, R1W), R1W)

    qqe_all = rows[:, 2 * F1 : 2 * F1 + QQE1]  # [NTOT, 644]
    s1_all = rows[:, s0 : s0 + F1]

    # ---------------- launch 1: edge phase 1 -> h ----------------
    if "e1" not in _built:
        _built["e1"] = _build_edge1()
    in_maps1 = []
    for c in range(NCORES):
        in_maps1.append({
            "qqe": np.ascontiguousarray(qqe_all[c * NLOC : (c + 1) * NLOC]),
            "s1": np.ascontiguousarray(s1_all[c * NLOC : (c + 1) * NLOC]),
            "R": R1p[c],
            "S": S_p[c], "ST": ST_p[c],
            "Wevp": Wevp_bf,
            "ident": ident,
        })
    r1 = _run(_built["e1"], in_maps1)
    hT_all = np.ascontiguousarray(
        np.concatenate([r1.results[c]["h"] for c in range(NCORES)], axis=0).T
    )  # [F1, NTOT] bf16

    # ---------------- launch 1b: layer-2 node GEMM ----------------
    if "g2" not in _built:
        _built["g2"] = _build_gemm2()
    in_maps1b = []
    for c in range(NCORES):
        in_maps1b.append({
            "hT": np.ascontiguousarray(hT_all[:, c * NLOC : (c + 1) * NLOC]),
            "W2a": W2all_bf,
            "b2T": b2T,
        })
    r1b = _run(_built["g2"], in_maps1b)
    out2_all = np.ascontiguousarray(
        np.concatenate([r1b.results[c]["out2T"] for c in range(NCORES)], axis=1).T
    )  # [NTOT, 289] bf16

    # host gather for layer 2
    src2 = out2_all[plan.edge_src_gslot.reshape(-1)]  # [cores*ELOC, 289]
    R2 = np.zeros((NCORES * ELOC, R2W), dtype=bf)
    R2[:, :OUT] = src2[:, :OUT]                 # k2
    R2[:, OUT:HW2] = eap.reshape(-1, EAW)       # ea'
    R2[:, HW2 : HW2 + OUT] = src2[:, OUT : 2 * OUT]  # v2
    R2[:, HW2 + OUT :] = eap.reshape(-1, EAW)
    R2p = _pack_rows_blocks(R2.reshape(NCORES, ELOC, R2W), R2W)

    qqe2_all = out2_all[:, 2 * OUT : 2 * OUT + HW2]
    s2_all = out2_all[:, 2 * OUT + HW2 :]

    # ---------------- launch 2: edge phase 2 ----------------
    if "e2" not in _built:
        _built["e2"] = _build_edge2()
    in_maps2 = []
    for c in range(NCORES):
        in_maps2.append({
            "qqe2": np.ascontiguousarray(qqe2_all[c * NLOC : (c + 1) * NLOC]),
            "s2": np.ascontiguousarray(s2_all[c * NLOC : (c + 1) * NLOC]),
            "R2": R2p[c],
            "S": S_p[c], "ST": ST_p[c],
            "Wev2": Wev2, "ident": ident,
        })
    r2 = _run(_built["e2"], in_maps2)
    z_all = np.concatenate([r2.results[c]["z"] for c in range(NCORES)], axis=0)

    z = z_all[plan.node_gslot]
    if PROFILE:
        LAST_EXEC_NS = sum(int(t) for t in _exec_ns if t) if all(_exec_ns) else None
        LAST_TRACES = _traces
    return z.astype(np.float32)


# revision 23
# speedup vs baseline: 1.4352x; 1.0081x over previous
"""TransformerConv 2-layer GNN encoder on 8 Trainium2 NeuronCores.

Strategy (dst-sharded graph parallelism, v2):
  - Nodes assigned to 8 cores x 20 blocks x 128 slots (degree-balanced FFD);
    each block has <= 1024 incoming edges -> 8 chunks of 128 edges.
  - Launch 0 (node1): feature-major GEMM nodeT = Wn^T @ x per core producing
    k|v|q|qe'|s rows, biases folded in via per-partition ACT/DVE bias.
    qe' = q @ [We_h | bk_h] folds the edge-attr term and k-bias of the
    attention logit into a 33-wide per-node vector (alpha = q.k + qe'.ea').
  - Host gathers per-edge source rows [k_h|ea']x4 | [v_h|ea']x4 (pure data
    movement between launches).
  - Launch 1 (edge1): per chunk: PE gathers [q|qe'] rows via one-hot ST
    matmuls, DVE tensor_tensor_reduce computes alpha per head straight from
    PSUM, ACT exponentiates and scales [v_h|ea'] by ex, PE scatters via
    one-hot S into numerator + 33-wide [ex*ea'|ex] accumulators. Block
    epilogue expands the ea'-accumulator through [We;bv] (the ones column
    doubles as softmax denominator and v-bias), normalizes, adds skip, relu,
    and runs the fused layer-2 node GEMM producing k2|v2|q2|qe2'|s2.
  - Launch 2 (edge2): same edge pipeline at width 64/33 -> z.
"""

import sys

sys.path.insert(0, "/opt/trn_rl_repo")

import json

import numpy as np

# ----------------------------------------------------------------------------
# Problem constants (hardcoded per contract)
# ----------------------------------------------------------------------------
N, E, IN_DIM, EDGE_DIM, HID, OUT = 20000, 160000, 128, 32, 128, 64
H1 = 4
F1 = H1 * HID  # 512
NCORES = 8
BLKS = 20          # dst blocks per core
BLKN = 128         # nodes per block
NLOC = BLKS * BLKN  # 2560 nodes per core
NTOT = NCORES * NLOC  # 20480 slots
CPB = 8            # chunks per block
T = 128            # edges per chunk
ECHUNKS = BLKS * CPB  # 160 chunks per core
ELOC = ECHUNKS * T    # 20480 edge slots per core

EAW = EDGE_DIM + 1          # 33: [ea | 1]
HW1 = HID + EAW             # 161: per-head [k|ea'] / [q|qe'] width
R1W = 2 * H1 * HW1          # 1288 gathered row width, layer 1
QQE1 = H1 * HW1             # 644
NF0 = 2304                  # node1 output rows (2180 used, padded to 18*128)
NCH0 = NF0 // 128           # 18

HW2 = OUT + EAW             # 97
R2W = 2 * HW2               # 194
O2W = 4 * OUT + EAW         # 289: k2|v2|q2|qe2'|s2

ISQ1 = 1.0 / np.sqrt(np.float32(HID))
ISQ2 = 1.0 / np.sqrt(np.float32(OUT))
DENOM_EPS = 1e-30

# ----------------------------------------------------------------------------
# Walrus single-wait shim + NTFF profiling hook (inlined; must be
# self-contained).
# ----------------------------------------------------------------------------
_shim_installed = False


def _split_waits_in_bir(bir_bytes: bytes) -> bytes:
    d = json.loads(bir_bytes)
    for fn in d.get("functions", []):
        for blk in fn.get("blocks", []):
            new_insts = []
            for ins in blk.get("instructions", []):
                si = ins.get("sync_info") or {}
                waits = si.get("on_wait") or []
                if len(waits) > 1:
                    for k, w in enumerate(waits[:-1]):
                        ev = {
                            "name": f"{ins['name']}_wsplit{k}",
                            "opcode": "EventSemaphore",
                            "engine": ins["engine"],
                            "ins": [],
                            "outs": [],
                            "sync_info": {"on_wait": [w], "on_update": []},
                        }
                        if "debug" in ins:
                            ev["debug"] = ins["debug"]
                        new_insts.append(ev)
                    si["on_wait"] = [waits[-1]]
                new_insts.append(ins)
            blk["instructions"] = new_insts
    return json.dumps(d).encode()


def _install_shim():
    global _shim_installed
    if _shim_installed:
        return
    import concourse.bass2jax as bass2jax
    import concourse.bass_utils as bass_utils

    orig = bass_utils.compile_bir_kernel

    def wrapped(bir_json, tmpdir, neff_name="file.neff"):
        if isinstance(bir_json, str):
            bir_json = bir_json.encode()
        return orig(_split_waits_in_bir(bir_json), tmpdir, neff_name=neff_name)

    bass_utils.compile_bir_kernel = wrapped
    bass2jax.compile_bir_kernel = wrapped

    import types

    try:
        from antenv import axon_hooks  # noqa: F401
    except ImportError:
        import antenv

        mod = types.ModuleType("antenv.axon_hooks")
        _state = {"hook": None}
        mod.set_axon_ntff_profile_hook = lambda h: _state.__setitem__("hook", h)
        mod.get_axon_ntff_profile_hook = lambda: _state["hook"]
        sys.modules["antenv.axon_hooks"] = mod
        antenv.axon_hooks = mod
        try:
            from trn_agent_boot.trn_boot import _ntff_profile_via_ctypes

            hook = _ntff_profile_via_ctypes("/opt/axon/libaxon_pjrt.so")
            if hook is not None:
                mod.set_axon_ntff_profile_hook(hook)
        except Exception:
            pass
    _shim_installed = True


# ----------------------------------------------------------------------------
# Host-side graph planning
# ----------------------------------------------------------------------------
class _Plan:
    pass


def _make_plan(ei: np.ndarray) -> _Plan:
    """Assign nodes to (core, block, slot); build per-core edge schedule."""
    src = np.asarray(ei[0], dtype=np.int64)
    dst = np.asarray(ei[1], dtype=np.int64)
    deg = np.bincount(dst, minlength=N)  # in-degree

    nbins = NCORES * BLKS  # 160
    cap_e = CPB * T  # 1024 edges per bin
    cap_n = BLKN  # 128 nodes per bin

    order = np.argsort(-deg, kind="stable")
    bin_e = np.zeros(nbins, dtype=np.int64)
    bin_n = np.zeros(nbins, dtype=np.int64)
    node_bin = np.empty(N, dtype=np.int64)
    start = 0
    for nd in order:
        d = deg[nd]
        placed = False
        for k in range(nbins):
            b = (start + k) % nbins
            if bin_e[b] + d <= cap_e and bin_n[b] < cap_n:
                node_bin[nd] = b
                bin_e[b] += d
                bin_n[b] += 1
                start = (b + 1) % nbins
                placed = True
                break
        if not placed:
            raise RuntimeError("bin packing failed")

    node_slot = np.empty(N, dtype=np.int64)
    fill = np.zeros(nbins, dtype=np.int64)
    for nd in range(N):
        b = node_bin[nd]
        node_slot[nd] = fill[b]
        fill[b] += 1

    node_gslot = node_bin * BLKN + node_slot  # in [0, NTOT)

    edge_bin = node_bin[dst]
    eorder = np.argsort(edge_bin, kind="stable")
    sorted_bins = edge_bin[eorder]
    bin_starts = np.searchsorted(sorted_bins, np.arange(nbins))
    bin_ends = np.searchsorted(sorted_bins, np.arange(nbins), side="right")

    edge_src_gslot = np.zeros((NCORES, ELOC), dtype=np.int64)
    edge_id = np.full((NCORES, ELOC), -1, dtype=np.int64)
    edge_dslot = np.full((NCORES, ELOC), -1, dtype=np.int64)
    for b in range(nbins):
        core = b // BLKS
        blk = b % BLKS
        s, e = bin_starts[b], bin_ends[b]
        eids = eorder[s:e]
        ne = len(eids)
        base = blk * cap_e
        edge_id[core, base : base + ne] = eids
        edge_src_gslot[core, base : base + ne] = node_gslot[src[eids]]
        edge_dslot[core, base : base + ne] = node_slot[dst[eids]]

    p = _Plan()
    p.node_gslot = node_gslot
    p.edge_src_gslot = edge_src_gslot
    p.edge_id = edge_id
    p.edge_dslot = edge_dslot
    return p


def _build_S_packed(plan):
    """Per-block packed one-hot matrices, bf16."""
    import ml_dtypes

    S = np.zeros((NCORES, ECHUNKS, T, BLKN), dtype=np.float32)
    dslot = plan.edge_dslot.reshape(NCORES, ECHUNKS, T)
    c_idx, ch_idx, t_idx = np.nonzero(dslot >= 0)
    S[c_idx, ch_idx, t_idx, dslot[c_idx, ch_idx, t_idx]] = 1.0
    bf = ml_dtypes.bfloat16
    S_p = np.ascontiguousarray(
        S.reshape(NCORES, BLKS, CPB, T, BLKN).transpose(0, 1, 3, 2, 4)
        .reshape(NCORES, BLKS, T, CPB * BLKN)
    ).astype(bf)
    ST_p = np.ascontiguousarray(
        S.reshape(NCORES, BLKS, CPB, T, BLKN).transpose(0, 1, 4, 2, 3)
        .reshape(NCORES, BLKS, BLKN, CPB * T)
    ).astype(bf)
    return S_p, ST_p


def _pack_rows_blocks(rows, width):
    """[cores, ELOC, width] -> [cores, BLKS, T, CPB*width] block-major."""
    return np.ascontiguousarray(
        rows.reshape(NCORES, BLKS, CPB, T, width).transpose(0, 1, 3, 2, 4)
        .reshape(NCORES, BLKS, T, CPB * width)
    )


# ----------------------------------------------------------------------------
# Bass kernel builders
# ----------------------------------------------------------------------------
_built = {}


def _get_nc():
    import concourse.bass as bass

    return bass.Bass(target_bir_lowering=False, trn_type="TRN2")


def _build_node1():
    """Feature-major node GEMM: nodeT[f, n] = sum_d Wn[d, f] x[n, d] + bias[f]."""
    import concourse.mybir as mybir
    from concourse.tile import TileContext

    dt = mybir.dt
    nc = _get_nc()
    f32, bf = dt.float32, dt.bfloat16
    AF = mybir.ActivationFunctionType

    xT = nc.dram_tensor("xT", [IN_DIM, NLOC], bf, kind="ExternalInput")
    Wn = nc.dram_tensor("Wn", [IN_DIM, NF0], bf, kind="ExternalInput")
    bT = nc.dram_tensor("bT", [128, NCH0], f32, kind="ExternalInput")
    nodeT = nc.dram_tensor("nodeT", [NF0, NLOC], bf, kind="ExternalOutput")

    with TileContext(nc) as tc:
        with (
            tc.tile_pool(name="const", bufs=1) as cpool,
            tc.tile_pool(name="ob", bufs=2) as opool,
            tc.tile_pool(name="ps", bufs=6, space="PSUM") as psp,
        ):
            w = cpool.tile([IN_DIM, NF0], bf)
            nc.sync.dma_start(w[:], Wn[:])
            bt = cpool.tile([128, NCH0], f32)
            nc.sync.dma_start(bt[:], bT[:])
            xfull = cpool.tile([IN_DIM, NLOC], bf)
            nc.sync.dma_start(xfull[:], xT[:])
            for fc in range(NCH0):
                wslice = w[:, fc * 128 : (fc + 1) * 128]
                ob = opool.tile([128, NLOC], bf, tag="ob")
                for g in range(NLOC // 512):
                    ps = psp.tile([128, 512], f32, tag="ps")
                    nc.tensor.matmul(
                        ps[:], wslice, xfull[:, g * 512 : (g + 1) * 512],
                        start=True, stop=True,
                    )
                    osl = ob[:, g * 512 : (g + 1) * 512]
                    if g % 2 == 1:
                        nc.vector.tensor_scalar_add(osl, ps[:], bt[:, fc : fc + 1])
                    else:
                        nc.scalar.activation(
                            osl, ps[:], AF.Identity, bias=bt[:, fc : fc + 1]
                        )
                nc.sync.dma_start(nodeT[fc * 128 : (fc + 1) * 128, :], ob[:])
    return nc


def _build_edge1():
    """Layer-1 edge phase -> h (relu'd layer-1 output), node-major."""
    import concourse.mybir as mybir
    from concourse.tile import TileContext

    dt = mybir.dt
    nc = _get_nc()
    f32, bf = dt.float32, dt.bfloat16
    AF = mybir.ActivationFunctionType
    ALU = mybir.AluOpType

    qqed = nc.dram_tensor("qqe", [NLOC, QQE1], bf, kind="ExternalInput")
    s1d = nc.dram_tensor("s1", [NLOC, F1], bf, kind="ExternalInput")
    Rd = nc.dram_tensor("R", [BLKS, T, CPB * R1W], bf, kind="ExternalInput")
    Sd = nc.dram_tensor("S", [BLKS, T, CPB * BLKN], bf, kind="ExternalInput")
    STd = nc.dram_tensor("ST", [BLKS, BLKN, CPB * T], bf, kind="ExternalInput")
    Wevd = nc.dram_tensor("Wevp", [2 * EAW, 2 * 256], bf, kind="ExternalInput")
    identd = nc.dram_tensor("ident", [BLKN, BLKN], bf, kind="ExternalInput")
    hd = nc.dram_tensor("h", [NLOC, F1], bf, kind="ExternalOutput")

    with TileContext(nc) as tc:
        with (
            tc.tile_pool(name="const", bufs=1) as cpool,
            tc.tile_pool(name="blk", bufs=3) as bpool,
            tc.tile_pool(name="sb", bufs=4) as pool,
            tc.tile_pool(name="epi", bufs=2) as epool,
            tc.tile_pool(name="psq", bufs=2, space="PSUM") as psq,
            tc.tile_pool(name="psacc", bufs=1, space="PSUM") as psa,
            tc.tile_pool(name="psepi", bufs=2, space="PSUM") as pse,
        ):
            wevp = cpool.tile([2 * EAW, 2 * 256], bf)
            nc.sync.dma_start(wevp[:], Wevd[:])
            ident = cpool.tile([BLKN, BLKN], bf)
            nc.sync.dma_start(ident[:], identd[:])

            for b in range(BLKS):
                qqe_b = bpool.tile([BLKN, QQE1], bf, tag="qqe_b")
                nc.sync.dma_start(qqe_b[:], qqed[b * BLKN : (b + 1) * BLKN, :])
                s1b = bpool.tile([BLKN, F1], bf, tag="s1b")
                nc.sync.dma_start(s1b[:], s1d[b * BLKN : (b + 1) * BLKN, :])
                Sb = bpool.tile([T, CPB * BLKN], bf, tag="Sb")
                nc.sync.dma_start(Sb[:], Sd[b])
                STb = bpool.tile([BLKN, CPB * T], bf, tag="STb")
                nc.sync.dma_start(STb[:], STd[b])
                Rb = bpool.tile([T, CPB * R1W], bf, tag="Rb")
                nc.sync.dma_start(Rb[:], Rd[b])

                psN = psa.tile([BLKN, F1], f32, tag="psN")
                psE = psa.tile([BLKN, H1 * EAW], f32, tag="psE")

                for i in range(CPB):
                    Rc = Rb[:, i * R1W : (i + 1) * R1W]
                    s_ = Sb[:, i * BLKN : (i + 1) * BLKN]
                    st_ = STb[:, i * T : (i + 1) * T]

                    qtA = psq.tile([T, 3 * HW1], f32, tag="qtA")
                    nc.tensor.matmul(
                        qtA[:], st_, qqe_b[:, : 3 * HW1], start=True, stop=True
                    )
                    qtB = psq.tile([T, HW1], f32, tag="qtB")
                    nc.tensor.matmul(
                        qtB[:], st_, qqe_b[:, 3 * HW1 : QQE1], start=True, stop=True
                    )

                    # copy gathered [q|qe'] to SBUF bf16 (ACT + DVE split)
                    qtqeb = pool.tile([T, QQE1], bf, tag="qtqeb")
                    nc.scalar.activation(qtqeb[:, : 3 * HW1], qtA[:], AF.Copy)
                    nc.vector.tensor_scalar_add(qtqeb[:, 3 * HW1 :], qtB[:], 0.0)

                    prod = pool.tile([T, QQE1], bf, tag="prod")
                    nc.vector.tensor_tensor(
                        prod[:], qtqeb[:], Rc[:, :QQE1], ALU.mult
                    )
                    alpha = pool.tile([T, H1], f32, tag="alpha")
                    nc.vector.tensor_reduce(
                        alpha[:],
                        prod[:].rearrange("p (h w) -> p h w", h=H1),
                        mybir.AxisListType.X,
                        ALU.add,
                    )
                    ex4 = pool.tile([T, H1], f32, tag="ex4")
                    nc.scalar.activation(ex4[:], alpha[:], AF.Exp, scale=ISQ1)

                    EXV = pool.tile([T, QQE1], bf, tag="EXV")
                    for h in range(H1):
                        vsl = Rc[:, QQE1 + h * HW1 : QQE1 + (h + 1) * HW1]
                        esl = EXV[:, h * HW1 : (h + 1) * HW1]
                        exh = ex4[:, h : h + 1]
                        if h == 1:
                            nc.vector.tensor_scalar_mul(esl, vsl, exh)
                        else:
                            nc.scalar.activation(esl, vsl, AF.Copy, scale=exh)
                    ev3 = EXV[:].rearrange("p (h w) -> p h w", h=H1)
                    nc.tensor.matmul(
                        psN[:], s_, ev3[:, :, :HID],
                        start=(i == 0), stop=False,
                    )
                    nc.tensor.matmul(
                        psE[:], s_, ev3[:, :, HID:],
                        start=(i == 0), stop=(i == CPB - 1),
                    )

                # ---- block epilogue ----
                eEb = epool.tile([BLKN, H1 * EAW], bf, tag="eEb")
                nc.scalar.activation(eEb[:], psE[:], AF.Copy)
                den = epool.tile([BLKN, H1], f32, tag="den")
                nc.vector.tensor_scalar_max(
                    den[:].rearrange("p (h o) -> p h o", o=1),
                    psE[:].rearrange("p (h w) -> p h w", h=H1)[:, :, EDGE_DIM : EDGE_DIM + 1],
                    DENOM_EPS,
                )
                rcp = epool.tile([BLKN, H1], f32, tag="rcp")
                nc.vector.reciprocal(rcp[:], den[:])

                for p in range(2):
                    pst = pse.tile([2 * EAW, BLKN], bf, tag="pst")
                    nc.tensor.transpose(
                        pst[:], eEb[:, p * 2 * EAW : (p + 1) * 2 * EAW], ident[:]
                    )
                    eET = epool.tile([2 * EAW, BLKN], bf, tag="eET")
                    nc.vector.tensor_scalar_add(eET[:], pst[:], 0.0)
                    nc.tensor.matmul(
                        psN[:, p * 256 : (p + 1) * 256],
                        eET[:],
                        wevp[:, p * 256 : (p + 1) * 256],
                        start=False,
                        stop=True,
                        skip_group_check=True,
                    )

                hb = epool.tile([BLKN, F1], bf, tag="hb")
                for h in range(H1):
                    nc.vector.scalar_tensor_tensor(
                        hb[:, h * HID : (h + 1) * HID],
                        psN[:, h * HID : (h + 1) * HID],
                        rcp[:, h : h + 1],
                        s1b[:, h * HID : (h + 1) * HID],
                        ALU.mult,
                        ALU.add,
                    )
                hr = epool.tile([BLKN, F1], bf, tag="hr")
                nc.vector.tensor_scalar_max(hr[:], hb[:], 0.0)
                nc.sync.dma_start(hd[b * BLKN : (b + 1) * BLKN, :], hr[:])
    return nc


def _build_gemm2():
    """Feature-major layer-2 node GEMM: out2T = W2cat @ h^T + b2."""
    import concourse.mybir as mybir
    from concourse.tile import TileContext

    dt = mybir.dt
    nc = _get_nc()
    f32, bf = dt.float32, dt.bfloat16
    AF = mybir.ActivationFunctionType

    hTd = nc.dram_tensor("hT", [F1, NLOC], bf, kind="ExternalInput")
    W2d = nc.dram_tensor("W2a", [128, 4 * O2W], bf, kind="ExternalInput")
    b2Td = nc.dram_tensor("b2T", [128, 3], f32, kind="ExternalInput")
    out2T = nc.dram_tensor("out2T", [O2W, NLOC], bf, kind="ExternalOutput")

    FCS = [(0, 128), (128, 256), (256, O2W)]
    with TileContext(nc) as tc:
        with (
            tc.tile_pool(name="const", bufs=1) as cpool,
            tc.tile_pool(name="ob", bufs=2) as opool,
            tc.tile_pool(name="ps", bufs=6, space="PSUM") as psp,
        ):
            w2a = cpool.tile([128, 4 * O2W], bf)
            nc.sync.dma_start(w2a[:], W2d[:])
            b2t = cpool.tile([128, 3], f32)
            nc.sync.dma_start(b2t[:], b2Td[:])
            hts = []
            for hk in range(4):
                ht = cpool.tile([128, NLOC], bf, name=f"ht{hk}")
                nc.sync.dma_start(ht[:], hTd[hk * 128 : (hk + 1) * 128, :])
                hts.append(ht)
            for ci, (f0, f1) in enumerate(FCS):
                fw = f1 - f0
                ob = opool.tile([128, NLOC], bf, tag="ob")
                for g in range(NLOC // 512):
                    ps = psp.tile([128, 512], f32, tag="ps")
                    for hk in range(4):
                        nc.tensor.matmul(
                            ps[:fw, :],
                            w2a[:, hk * O2W + f0 : hk * O2W + f1],
                            hts[hk][:, g * 512 : (g + 1) * 512],
                            start=(hk == 0), stop=(hk == 3),
                        )
                    osl = ob[:fw, g * 512 : (g + 1) * 512]
                    if g % 3 == 2:
                        nc.vector.tensor_scalar_add(
                            osl, ps[:fw, :], b2t[:fw, ci : ci + 1]
                        )
                    else:
                        nc.scalar.activation(
                            osl, ps[:fw, :], AF.Identity,
                            bias=b2t[:fw, ci : ci + 1],
                        )
                nc.sync.dma_start(out2T[f0:f1, :], ob[:fw, :])
    return nc


def _build_edge2():
    """Layer-2 edge phase -> z."""
    import concourse.mybir as mybir
    from concourse.tile import TileContext

    dt = mybir.dt
    nc = _get_nc()
    f32, bf = dt.float32, dt.bfloat16
    AF = mybir.ActivationFunctionType
    ALU = mybir.AluOpType

    qqed = nc.dram_tensor("qqe2", [NLOC, HW2], bf, kind="ExternalInput")
    s2d = nc.dram_tensor("s2", [NLOC, OUT], bf, kind="ExternalInput")
    Rd = nc.dram_tensor("R2", [BLKS, T, CPB * R2W], bf, kind="ExternalInput")
    Sd = nc.dram_tensor("S", [BLKS, T, CPB * BLKN], bf, kind="ExternalInput")
    STd = nc.dram_tensor("ST", [BLKS, BLKN, CPB * T], bf, kind="ExternalInput")
    Wevd = nc.dram_tensor("Wev2", [EAW, OUT], bf, kind="ExternalInput")
    identd = nc.dram_tensor("ident", [BLKN, BLKN], bf, kind="ExternalInput")
    zd = nc.dram_tensor("z", [NLOC, OUT], f32, kind="ExternalOutput")

    with TileContext(nc) as tc:
        with (
            tc.tile_pool(name="const", bufs=1) as cpool,
            tc.tile_pool(name="blk", bufs=2) as bpool,
            tc.tile_pool(name="sb", bufs=3) as pool,
            tc.tile_pool(name="epi", bufs=2) as epool,
            tc.tile_pool(name="psq", bufs=2, space="PSUM") as psq,
            tc.tile_pool(name="psacc", bufs=1, space="PSUM") as psa,
            tc.tile_pool(name="psepi", bufs=1, space="PSUM") as pse,
        ):
            wev = cpool.tile([EAW, OUT], bf)
            nc.sync.dma_start(wev[:], Wevd[:])
            ident = cpool.tile([BLKN, BLKN], bf)
            nc.sync.dma_start(ident[:], identd[:])

            for b in range(BLKS):
                qqe_b = bpool.tile([BLKN, HW2], bf, tag="qqe_b")
                nc.sync.dma_start(qqe_b[:], qqed[b * BLKN : (b + 1) * BLKN, :])
                s2b = bpool.tile([BLKN, OUT], bf, tag="s2b")
                nc.sync.dma_start(s2b[:], s2d[b * BLKN : (b + 1) * BLKN, :])
                Sb = bpool.tile([T, CPB * BLKN], bf, tag="Sb")
                nc.sync.dma_start(Sb[:], Sd[b])
                STb = bpool.tile([BLKN, CPB * T], bf, tag="STb")
                nc.sync.dma_start(STb[:], STd[b])
                Rb = bpool.tile([T, CPB * R2W], bf, tag="Rb")
                nc.sync.dma_start(Rb[:], Rd[b])

                psX = psa.tile([BLKN, HW2], f32, tag="psX")

                for i in range(CPB):
                    Rc = Rb[:, i * R2W : (i + 1) * R2W]
                    s_ = Sb[:, i * BLKN : (i + 1) * BLKN]
                    st_ = STb[:, i * T : (i + 1) * T]

                    qt = psq.tile([T, HW2], f32, tag="qt")
                    nc.tensor.matmul(qt[:], st_, qqe_b[:], start=True, stop=True)

                    prod = pool.tile([T, HW2], bf, tag="prod")
                    nc.vector.tensor_tensor(
                        prod[:], qt[:], Rc[:, :HW2], ALU.mult
                    )
                    alpha = pool.tile([T, 1], f32, tag="alpha")
                    nc.vector.tensor_reduce(
                        alpha[:], prod[:], mybir.AxisListType.X, ALU.add
                    )
                    ex = pool.tile([T, 1], f32, tag="ex")
                    nc.scalar.activation(ex[:], alpha[:], AF.Exp, scale=ISQ2)

                    EXV = pool.tile([T, HW2], bf, tag="EXV")
                    nc.vector.tensor_scalar_mul(EXV[:], Rc[:, HW2:R2W], ex[:])
                    nc.tensor.matmul(
                        psX[:], s_, EXV[:], start=(i == 0), stop=(i == CPB - 1)
                    )

                # ---- block epilogue ----
                eEb = epool.tile([BLKN, EAW], bf, tag="eEb")
                nc.scalar.activation(eEb[:], psX[:, OUT:HW2], AF.Copy)
                den = epool.tile([BLKN, 1], f32, tag="den")
                nc.vector.tensor_scalar_max(
                    den[:], psX[:, HW2 - 1 : HW2], DENOM_EPS
                )
                rcp = epool.tile([BLKN, 1], f32, tag="rcp")
                nc.vector.reciprocal(rcp[:], den[:])

                pst = pse.tile([EAW, BLKN], bf, tag="pst")
                nc.tensor.transpose(pst[:], eEb[:], ident[:])
                eET = epool.tile([EAW, BLKN], bf, tag="eET")
                nc.vector.tensor_scalar_add(eET[:], pst[:], 0.0)
                nc.tensor.matmul(
                    psX[:, :OUT], eET[:], wev[:],
                    start=False, stop=True, skip_group_check=True,
                )

                zt = epool.tile([BLKN, OUT], f32, tag="zt")
                nc.vector.scalar_tensor_tensor(
                    zt[:], psX[:, :OUT], rcp[:], s2b[:], ALU.mult, ALU.add
                )
                nc.sync.dma_start(zd[b * BLKN : (b + 1) * BLKN, :], zt[:])
    return nc


# ----------------------------------------------------------------------------
# Kernel entry point
# ----------------------------------------------------------------------------
PROFILE = False
LAST_EXEC_NS = None
LAST_TRACES = None


def kernel(**inputs):
    global LAST_EXEC_NS, LAST_TRACES
    _install_shim()
    import ml_dtypes

    from concourse import bass_utils

    bf = ml_dtypes.bfloat16

    def _run(nc, in_maps):
        r = bass_utils.run_bass_kernel_spmd(
            nc, in_maps, core_ids=list(range(NCORES)), trace=PROFILE
        )
        if PROFILE:
            _exec_ns.append(r.exec_time_ns)
            _traces.append(r.instructions_and_trace)
        return r

    _exec_ns, _traces = [], []

    x = np.asarray(inputs["x"], dtype=np.float32)
    ei = np.asarray(inputs["ei"])
    ea = np.asarray(inputs["ea"], dtype=np.float32)
    W = {k: np.asarray(v, dtype=np.float32) for k, v in inputs.items()
         if k not in ("x", "ei", "ea")}

    plan = _make_plan(ei)
    S_p, ST_p = _build_S_packed(plan)

    # per-edge [ea | 1] rows (zeros on padding slots)
    eid = plan.edge_id  # [cores, ELOC]
    valid = eid >= 0
    eap = np.zeros((NCORES, ELOC, EAW), dtype=np.float32)
    eap[valid, :EDGE_DIM] = ea[eid[valid]]
    eap[valid, EDGE_DIM] = 1.0
    eap = eap.astype(bf)

    # node features in slot order
    x_slots = np.zeros((NTOT, IN_DIM), dtype=np.float32)
    x_slots[plan.node_gslot] = x
    xT_all = np.ascontiguousarray(x_slots.T).astype(bf)  # [128, NTOT]

    # ------------- weight folding (host, f32) -------------
    Wq1, bq1 = W["Wq1"], W["bq1"]
    Wk1, bk1 = W["Wk1"], W["bk1"]
    Wv1, bv1 = W["Wv1"], W["bv1"]
    We1, Ws1, bs1 = W["We1"], W["Ws1"], W["bs1"]

    Wn = np.zeros((IN_DIM, NF0), dtype=np.float32)
    bn = np.zeros(NF0, dtype=np.float32)
    Wn[:, 0:F1] = Wk1.T          # k (no bias; bk rides qe' ones col)
    Wn[:, F1 : 2 * F1] = Wv1.T   # v (no bias; bv rides Wev ones row)
    for h in range(H1):
        Wq_h = Wq1[h * HID : (h + 1) * HID]       # [128, 128]
        bq_h = bq1[h * HID : (h + 1) * HID]
        We_h = We1[h * HID : (h + 1) * HID]       # [128, 32]
        bk_h = bk1[h * HID : (h + 1) * HID]
        M_h = np.concatenate([We_h, bk_h[:, None]], axis=1)  # [128, 33]
        c0 = 2 * F1 + h * HW1
        Wn[:, c0 : c0 + HID] = Wq_h.T
        Wn[:, c0 + HID : c0 + HW1] = Wq_h.T @ M_h
        bn[c0 : c0 + HID] = bq_h
        bn[c0 + HID : c0 + HW1] = bq_h @ M_h
    s0 = 2 * F1 + QQE1
    Wn[:, s0 : s0 + F1] = Ws1.T
    bn[s0 : s0 + F1] = bs1
    Wn_bf = Wn.astype(bf)
    bT = np.ascontiguousarray(bn.reshape(NCH0, 128).T)  # [128, 18] f32

    # Wevp: [66, 512]; head-pair block-diagonal [We_h^T; bv_h] expansions
    Wevp = np.zeros((2 * EAW, 2 * 256), dtype=np.float32)
    for h in range(H1):
        p, q = divmod(h, 2)
        r0, c0 = q * EAW, p * 256 + q * HID
        Wevp[r0 : r0 + EDGE_DIM, c0 : c0 + HID] = We1[h * HID : (h + 1) * HID].T
        Wevp[r0 + EDGE_DIM, c0 : c0 + HID] = bv1[h * HID : (h + 1) * HID]
    Wevp_bf = Wevp.astype(bf)

    # layer-2 node GEMM weights: rows [Wk2|Wv2|Wq2|Wqe2'|Ws2] of [O2W, 512]
    Wq2, bq2 = W["Wq2"], W["bq2"]
    Wk2, bk2 = W["Wk2"], W["bk2"]
    Wv2, bv2 = W["Wv2"], W["bv2"]
    We2, Ws2, bs2 = W["We2"], W["Ws2"], W["bs2"]
    M2 = np.concatenate([We2, bk2[:, None]], axis=1)  # [64, 33]
    W2cat = np.concatenate(
        [Wk2, Wv2, Wq2, M2.T @ Wq2, Ws2], axis=0
    )  # [289, 512]
    b2row = np.concatenate(
        [np.zeros(OUT), np.zeros(OUT), bq2, bq2 @ M2, bs2]
    ).astype(np.float32)  # [289]
    W2all = np.zeros((128, 4 * O2W), dtype=np.float32)
    for fb in range(4):
        W2all[:, fb * O2W : (fb + 1) * O2W] = W2cat[:, fb * 128 : (fb + 1) * 128].T
    W2all_bf = W2all.astype(bf)
    b2pad = np.zeros(384, dtype=np.float32)
    b2pad[:O2W] = b2row
    b2T = np.ascontiguousarray(b2pad.reshape(3, 128).T)  # [128, 3] f32

    Wev2 = np.concatenate([We2.T, bv2[None, :]], axis=0).astype(bf)  # [33, 64]

    ident = np.eye(BLKN, dtype=np.float32).astype(bf)
    ones = np.ones((1, BLKN), dtype=np.float32).astype(bf)

    # ---------------- launch 0: node phase ----------------
    if "n1" not in _built:
        _built["n1"] = _build_node1()
    in_maps0 = []
    for c in range(NCORES):
        in_maps0.append({
            "xT": np.ascontiguousarray(xT_all[:, c * NLOC : (c + 1) * NLOC]),
            "Wn": Wn_bf,
            "bT": bT,
        })
    r0 = _run(_built["n1"], in_maps0)
    nodeT_all = np.concatenate(
        [r0.results[c]["nodeT"] for c in range(NCORES)], axis=1
    )  # [NF0, NTOT] bf16
    rows = np.ascontiguousarray(nodeT_all.T)  # [NTOT, NF0]

    # host gather: per-edge R rows [[k_h|ea']x4 | [v_h|ea']x4]
    src_rows = rows[plan.edge_src_gslot.reshape(-1)]  # [cores*ELOC, NF0]
    kv = src_rows[:, : 2 * F1].reshape(-1, 2, H1, HID)
    R1 = np.zeros((NCORES * ELOC, 2, H1, HW1), dtype=bf)
    R1[:, :, :, :HID] = kv
    R1[:, :, :, HID:] = eap.reshape(-1, EAW)[:, None, None, :]
    R1p = _pack_rows_blocks(R1.reshape(NCORES, ELOC, R1W), R1W)

    qqe_all = rows[:, 2 * F1 : 2 * F1 + QQE1]  # [NTOT, 644]
    s1_all = rows[:, s0 : s0 + F1]

    # ---------------- launch 1: edge phase 1 -> h ----------------
    if "e1" not in _built:
        _built["e1"] = _build_edge1()
    in_maps1 = []
    for c in range(NCORES):
        in_maps1.append({
            "qqe": np.ascontiguousarray(qqe_all[c * NLOC : (c + 1) * NLOC]),
            "s1": np.ascontiguousarray(s1_all[c * NLOC : (c + 1) * NLOC]),
            "R": R1p[c],
            "S": S_p[c], "ST": ST_p[c],
            "Wevp": Wevp_bf,
            "ident": ident,
        })
    r1 = _run(_built["e1"], in_maps1)
    hT_all = np.ascontiguousarray(
        np.concatenate([r1.results[c]["h"] for c in range(NCORES)], axis=0).T
    )  # [F1, NTOT] bf16

    # ---------------- launch 1b: layer-2 node GEMM ----------------
    if "g2" not in _built:
        _built["g2"] = _build_gemm2()
    in_maps1b = []
    for c in range(NCORES):
        in_maps1b.append({
            "hT": np.ascontiguousarray(hT_all[:, c * NLOC : (c + 1) * NLOC]),
            "W2a": W2all_bf,
            "b2T": b2T,
        })
    r1b = _run(_built["g2"], in_maps1b)
    out2_all = np.ascontiguousarray(
        np.concatenate([r1b.results[c]["out2T"] for c in range(NCORES)], axis=1).T
    )  # [NTOT, 289] bf16

    # host gather for layer 2
    src2 = out2_all[plan.edge_src_gslot.reshape(-1)]  # [cores*ELOC, 289]
    R2 = np.zeros((NCORES * ELOC, R2W), dtype=bf)
    R2[:, :OUT] = src2[:, :OUT]                 # k2
    R2[:, OUT:HW2] = eap.reshape(-1, EAW)       # ea'
    R2[:, HW2 : HW2 + OUT] = src2[:, OUT : 2 * OUT]  # v2
    R2[:, HW2 + OUT :] = eap.reshape(-1, EAW)
    R2p = _pack_rows_blocks(R2.reshape(NCORES, ELOC, R2W), R2W)

    qqe2_all = out2_all[:, 2 * OUT : 2 * OUT + HW2]
    s2_all = out2_all[:, 2 * OUT + HW2 :]

    # ---------------- launch 2: edge phase 2 ----------------
    if "e2" not in _built:
        _built["e2"] = _build_edge2()
    in_maps2 = []
    for c in range(NCORES):
        in_maps2.append({
            "qqe2": np.ascontiguousarray(qqe2_all[c * NLOC : (c + 1) * NLOC]),
            "s2": np.ascontiguousarray(s2_all[c * NLOC : (c + 1) * NLOC]),
            "R2": R2p[c],
            "S": S_p[c], "ST": ST_p[c],
            "Wev2": Wev2, "ident": ident,
        })
    r2 = _run(_built["e2"], in_maps2)
    z_all = np.concatenate([r2.results[c]["z"] for c in range(NCORES)], axis=0)

    z = z_all[plan.node_gslot]
    if PROFILE:
        LAST_EXEC_NS = sum(int(t) for t in _exec_ns if t) if all(_exec_ns) else None
        LAST_TRACES = _traces
    return z.astype(np.float32)
